# revision 1
# baseline (speedup 1.0000x reference)
"""Trainium2 Bass kernel for the DDSP decoder (nn_DDSPDecoder).

Sharding: pure data-parallel over batch B=8 across 8 NeuronCores; control-net
weights and reverb IR replicated; each core synthesizes one example.

Per-core pipeline (frames padded 1000->1024; GRU solved by quasi-linear Picard
iterations whose linear part is the hardware scan; harmonic phase factored
through the fundamental with a compensated two-float frame scan + Cody-Waite
range reduction; filtered noise done in the frequency domain with constant
DFT matrices; reverb as a 131072-pt 4-step FFT convolution on the PE).
"""
import os
import sys

if "/opt/trn_rl_repo" not in sys.path:
    sys.path.insert(0, "/opt/trn_rl_repo")

import numpy as np

import concourse.bass as bass  # noqa: F401
import concourse.tile as tile
from concourse import bacc, bass_utils, mybir
from concourse.alu_op_type import AluOpType as OP

F32 = mybir.dt.float32
F32R = mybir.dt.float32r
F16 = mybir.dt.bfloat16
I32 = mybir.dt.int32
AFT = mybir.ActivationFunctionType

SR = 16000
T = 1000
TP = 1024
NCH = 8                      # frame chunks of 128
NS = 64000
NSP = 65536
H = 64
NB = 65
N_CORES = 8
GRU_ITERS = 5
LN10 = float(np.float32(np.log(np.float32(10.0))))
LN2 = float(np.float32(np.log(2.0)))
TWO_PI = 2.0 * np.pi
TWOPI_F32 = float(np.float32(TWO_PI))
INV2PI = float(np.float32(1.0 / TWO_PI))
AX = mybir.AxisListType.X
MAGIC = 12582912.0  # 1.5 * 2**23: float32 round-to-nearest-int trick

DEBUG_TAPS = bool(int(os.environ.get("KERNEL_DEBUG_TAPS", "0")))


# ---------------------------------------------------------------------------
# host-side constants
# ---------------------------------------------------------------------------

def _cw_chunks(kmax_bits):
    nb = 24 - kmax_bits
    x = TWO_PI
    cs = []
    for _ in range(2):
        m, e = np.frexp(x)
        q = float(np.float32(np.ldexp(np.round(np.ldexp(m, nb)), e - nb)))
        cs.append(q)
        x -= q
    cs.append(float(np.float32(x)))
    return cs

CW_FRAME = _cw_chunks(12)
CW_SAMP = _cw_chunks(9)


def _noise_matrices():
    n = 128
    t = np.arange(n)
    C = np.zeros((NB, n))
    C[0] = 1.0 / n
    for m in range(1, 64):
        C[m] = 2.0 / n * np.cos(2 * np.pi * m * t / n)
    C[64] = 1.0 / n * np.cos(np.pi * t)
    w = 0.5 - 0.5 * np.cos(2.0 * np.pi * t / n)
    Cw = C * np.fft.fftshift(w)[None, :]
    Cc = np.zeros_like(Cw)
    Cc[:, (t + n // 2) % n] = Cw
    F = np.exp(-2j * np.pi * np.outer(t, np.arange(129)) / 256.0)
    Gc = Cc @ F
    Ff = np.exp(-2j * np.pi * np.outer(np.arange(64), np.arange(129)) / 256.0)
    tt = np.arange(256)
    DIre = np.zeros((129, 256))
    DIim = np.zeros((129, 256))
    for f in range(129):
        mult = 1.0 if f in (0, 128) else 2.0
        DIre[f] = mult * np.cos(2 * np.pi * f * tt / 256.0) / 256.0
        DIim[f] = -mult * np.sin(2 * np.pi * f * tt / 256.0) / 256.0
    return Gc, Ff, DIre, DIim


def _make_consts():
    f32 = np.float32
    c = {}
    j = np.arange(64, dtype=np.float64)
    sc = TWO_PI / SR
    c["w0ts"] = np.tile(f32((1.0 - j / 64.0) * sc), (128, 1))
    c["w1ts"] = np.tile(f32((j / 64.0) * sc), (128, 1))
    c["w0j"] = np.tile(f32(1.0 - j / 64.0), (128, 1))
    c["w1j"] = np.tile(f32(j / 64.0), (128, 1))
    h = np.arange(1, H + 1, dtype=np.float64)
    c["hrow"] = np.tile(f32(h), (128, 1))
    c["pioh"] = np.tile(f32(np.pi / h), (128, 1))
    c["c8000oh"] = np.tile(f32(8000.0 / h), (128, 1))
    c["eye"] = np.eye(128, dtype=np.float32)
    c["ones"] = np.ones((128, TP), np.float32)
    Gc, Ff, DIre, DIim = _noise_matrices()
    c["g_re"], c["g_im"] = f32(Gc.real), f32(Gc.imag)          # [65,129]
    c["d_re"], c["d_im"] = f32(Ff.real), f32(Ff.imag)          # [64,129]
    c["di_re"], c["di_im"] = f32(DIre), f32(DIim)              # [129,256]
    n1 = np.arange(256)
    k1 = np.arange(256)
    n2 = np.arange(512)
    W256 = np.exp(-2j * np.pi * np.outer(n1, k1) / 256.0)
    W512 = np.exp(-2j * np.pi * np.outer(n2, n2) / 512.0)
    TWf = np.exp(-2j * np.pi * np.outer(k1, n2) / 131072.0)
    TWi = np.exp(2j * np.pi * np.outer(n2, k1) / 131072.0)
    isc = 0.25 / 131072.0
    c["w256c"], c["w256s"] = f32(W256.real), f32(W256.imag)
    c["w512c"], c["w512s"] = f32(W512.real), f32(W512.imag)
    c["tw2c"] = f32(np.cos(2 * np.pi * n2 / 512.0)).reshape(512, 1)
    c["tw2s"] = f32(np.sin(2 * np.pi * n2 / 512.0)).reshape(512, 1)
    c["twfc"], c["twfs"] = f32(TWf.real), f32(TWf.imag)        # [256,512]
    c["twic"], c["twis"] = f32(TWi.real), f32(TWi.imag)        # [512,256]
    c["i2c"] = f32(np.cos(2 * np.pi * np.outer(k1, n1) / 256.0) * isc)   # [k1,n1]
    c["i2s"] = f32(-np.sin(2 * np.pi * np.outer(k1, n1) / 256.0) * isc)
    hbd = np.zeros((6, 128))
    for comp in range(3):
        hbd[comp, 0:64] = np.arange(1, 65)
        hbd[3 + comp, 64:128] = np.arange(1, 65)
    c["hbd6"] = f32(hbd)
    red = np.zeros((128, 8))
    red[0:64, 0] = 1.0    # mC lower strip -> r0 of chunk c0
    red[64:128, 2] = 1.0  # mC upper strip -> r0 of chunk c1
    red[0:64, 5] = 1.0    # mN lower strip -> r1 of chunk c0
    red[64:128, 7] = 1.0  # mN upper strip -> r1 of chunk c1
    c["redbd"] = f32(red)
    return {k: np.ascontiguousarray(v, np.float32) for k, v in c.items()}

CONSTS = _make_consts()


def _prep_weights(I):
    f32 = lambda v: np.ascontiguousarray(v, np.float32)
    d = {}
    d["pre_w"] = f32(I["pre_w"])                       # [2,128]
    d["pre_b"] = f32(np.reshape(I["pre_b"], (128, 1)))
    d["gru_k"] = f32(I["gru_k"])                       # [128,768]
    d["gru_rk"] = f32(I["gru_rk"])                     # [256,768]
    d["b0col"] = f32(np.reshape(I["gru_b"][0], (768, 1)))
    d["b1col"] = f32(np.reshape(I["gru_b"][1], (768, 1)))
    d["post_w1"] = f32(I["post_w1"])
    d["pb1col"] = f32(np.reshape(I["post_b1"], (256, 1)))
    d["post_w2"] = f32(I["post_w2"])
    d["pb2col"] = f32(np.reshape(I["post_b2"], (128, 1)))
    hw = np.zeros((128, 256), np.float32)
    hw[:, 0:1] = I["amp_w"]
    hw[:, 1:65] = I["harm_w"]
    hw[:, 65:130] = I["noise_w"]
    d["headsW"] = hw
    hb = np.zeros((256, 1), np.float32)
    hb[0, 0] = np.asarray(I["amp_b"]).reshape(-1)[0]
    hb[1:65, 0] = I["harm_b"]
    hb[65:130, 0] = I["noise_b"]
    d["headsB"] = hb
    return d

W_SHAPES = {
    "pre_w": (2, 128), "pre_b": (128, 1), "gru_k": (128, 768),
    "gru_rk": (256, 768), "b0col": (768, 1), "b1col": (768, 1),
    "post_w1": (256, 256), "pb1col": (256, 1), "post_w2": (256, 128),
    "pb2col": (128, 1), "headsW": (128, 256), "headsB": (256, 1),
}


# ---------------------------------------------------------------------------
# program builder
# ---------------------------------------------------------------------------

class Prog:
    def __init__(self, debug_taps=False):
        self.debug = debug_taps
        self.taps = {}
        nc = bacc.Bacc("TRN2", num_devices=N_CORES, target_bir_lowering=False,
                       debug=False)
        self.nc = nc
        self.din = {}
        for name, shape in [("f0pad", [TP + 32]), ("loudpad", [TP + 32]),
                            ("noisepad", [NSP]), ("rirpad", [NSP])]:
            self.din[name] = nc.dram_tensor(name, shape, F32, kind="ExternalInput")
        for k, s in W_SHAPES.items():
            self.din["w_" + k] = nc.dram_tensor("w_" + k, list(s), F32,
                                                kind="ExternalInput")
        for k, v in CONSTS.items():
            self.din["c_" + k] = nc.dram_tensor("c_" + k, list(v.shape), F32,
                                                kind="ExternalInput")
        self.out = nc.dram_tensor("audio_out", [NS], F32, kind="ExternalOutput")
        self.scr = {}
        for name, shape in [("a_dram", [TP + 8, 64]), ("mags_dram", [TP, NB]),
                            ("delta_dram", [TP]),
                            ("phired_dram", [TP]), ("audio_dram", [NSP]),
                            ("rh_dram", [2 * NCH * 8192]),
                            ("sq_re", [512, 256]), ("sq_im", [512, 256]),
                            ("of_dram", [TP + 8, 256])]:
            self.scr[name] = nc.dram_tensor(name, shape, F32)
        self.aT = nc.dram_tensor("aT_dram", [64, TP + 8], F16)
        self.phiC = [nc.dram_tensor(f"phi{i}_dram", [TP * 64], F16)
                     for i in range(3)]

    def tap(self, name, ap):
        if not self.debug:
            return
        t = self.nc.dram_tensor("tap_" + name, list(ap.shape), ap.dtype,
                                kind="ExternalOutput")
        self.nc.sync.dma_start(t.ap(), ap)
        self.taps[name] = t

    def build(self):
        nc = self.nc
        with tile.TileContext(nc) as tc:
            self._body(tc)
        nc.compile()
        return nc

    # -------------------------------------------------------------------
    def _body(self, tc):
        from contextlib import ExitStack
        with ExitStack() as ctx:
            self._phase_a(tc, ctx)
        with ExitStack() as ctx:
            self._phase_b(tc, ctx)
        with ExitStack() as ctx:
            self._phase_c(tc, ctx)

    # -------------------------------------------------------------------
    def _phase_a(self, tc, ctx):
        nc = self.nc
        din = self.din
        wp = ctx.enter_context(tc.tile_pool(name="wp", bufs=1))
        ap_ = ctx.enter_context(tc.tile_pool(name="actv", bufs=1))
        tp = ctx.enter_context(tc.tile_pool(name="tmpA", bufs=2))
        pp = ctx.enter_context(tc.tile_pool(name="psA", bufs=4, space="PSUM"))

        def wload(name, rows, cols, dtype=F32, rnd=False):
            """Load weight/const dram tensor into <=128-row SBUF tiles."""
            tiles = []
            nchunk = (rows + 127) // 128
            for i in range(nchunk):
                r0, r1 = i * 128, min((i + 1) * 128, rows)
                tl = wp.tile([r1 - r0, cols], dtype, name=f"{name}_{i}")
                if rnd:
                    st = tp.tile([r1 - r0, cols], F32, name=f"{name}_st{i}",
                                 tag="wstage")
                    nc.sync.dma_start(st[:], din[name].ap()[r0:r1])
                    nc.vector.tensor_copy(tl[:], st[:])
                else:
                    nc.sync.dma_start(tl[:], din[name].ap()[r0:r1])
                tiles.append(tl)
            return tiles

        prew_s0 = tp.tile([1, 128], F32, name="prew_s0")
        prew_s1 = tp.tile([1, 128], F32, name="prew_s1")
        nc.sync.dma_start(prew_s0[:], din["w_pre_w"].ap()[0:1])
        nc.sync.dma_start(prew_s1[:], din["w_pre_w"].ap()[1:2])
        prew0 = wp.tile([1, 128], F32R, name="prew0")
        prew1 = wp.tile([1, 128], F32R, name="prew1")
        nc.vector.tensor_copy(prew0[:], prew_s0[:])
        nc.vector.tensor_copy(prew1[:], prew_s1[:])
        pre_b = wload("w_pre_b", 128, 1)[0]
        gru_k = wload("w_gru_k", 128, 768, F32R, rnd=True)[0]
        rk = wload("w_gru_rk", 256, 768, F32R, rnd=True)
        b0 = wload("w_b0col", 768, 1)
        b1 = wload("w_b1col", 768, 1)
        pw1 = wload("w_post_w1", 256, 256, F32R, rnd=True)
        pb1 = wload("w_pb1col", 256, 1)
        pw2 = wload("w_post_w2", 256, 128, F32R, rnd=True)
        pb2 = wload("w_pb2col", 128, 1)[0]
        hW = wload("w_headsW", 128, 256, F32R, rnd=True)[0]
        hB = wload("w_headsB", 256, 1)
        eye = wload("c_eye", 128, 128)[0]
        c8000oh = wload("c_c8000oh", 128, 64)[0]

        ln2col = wp.tile([128, 1], F32, name="ln2col")
        nc.vector.memset(ln2col[:], LN2)
        m5col = wp.tile([128, 1], F32, name="m5col")
        nc.vector.memset(m5col[:], -5.0)

        # ---- f0m / loud rows [1, TP] ----
        f0row = tp.tile([1, TP], F32, name="f0row")
        loudrow = tp.tile([1, TP], F32, name="loudrow")
        nc.sync.dma_start(f0row[:], din["f0pad"].ap()[0:TP].unsqueeze(0))
        nc.sync.dma_start(loudrow[:], din["loudpad"].ap()[0:TP].unsqueeze(0))
        lnf0 = tp.tile([1, TP], F32, name="lnf0")
        nc.scalar.activation(lnf0[:], f0row[:], AFT.Ln)
        a_log = 12.0 / float(np.log(2.0))
        c_log = 69.0 - 12.0 * float(np.log2(440.0))
        f0mR = ap_.tile([1, TP], F32R, name="f0mR")
        loudR = ap_.tile([1, TP], F32R, name="loudR")
        nc.scalar.activation(f0mR[:], lnf0[:], AFT.Copy, scale=a_log,
                             bias=c_log)
        nc.scalar.activation(loudR[:], loudrow[:], AFT.Copy)

        # ---- x1T [128, TP] ----
        x1T = ap_.tile([128, TP], F32R, name="x1T")
        for n in range(2):
            ps = pp.tile([128, 512], F32, name="ps_x1", tag="psa")
            nc.tensor.matmul(ps[:], prew0[:], f0mR[:, n * 512:(n + 1) * 512],
                             start=True, stop=False)
            nc.tensor.matmul(ps[:], prew1[:], loudR[:, n * 512:(n + 1) * 512],
                             start=False, stop=True)
            nc.scalar.activation(x1T[:, n * 512:(n + 1) * 512], ps[:],
                                 AFT.Relu, bias=pre_b[:])
        self.tap("x1T", x1T[:])

        # ---- xpT 6 x [128, TP] ----
        xpT = []
        for m in range(6):
            xm = ap_.tile([128, TP], F32, name=f"xpT{m}")
            for n in range(2):
                ps = pp.tile([128, 512], F32, name="ps_xp", tag="psa")
                nc.tensor.matmul(ps[:], gru_k[:, m * 128:(m + 1) * 128],
                                 x1T[:, n * 512:(n + 1) * 512],
                                 start=True, stop=True)
                nc.scalar.activation(xm[:, n * 512:(n + 1) * 512], ps[:],
                                     AFT.Identity, bias=b0[m][:])
            xpT.append(xm)

        # ---- GRU Picard ----
        zcol = wp.tile([128, 1], F32, name="zcol")
        nc.vector.memset(zcol[:], 0.0)
        hT = []
        for k in range(2):
            ht = ap_.tile([128, TP + 1], F32R, name=f"hT{k}")
            nc.vector.tensor_copy(ht[:, 0:1], zcol[:])
            hT.append(ht)

        gp = ctx.enter_context(tc.tile_pool(name="gru", bufs=1))
        for it in range(GRU_ITERS):
            rpT = []
            if it > 0:
                for m in range(6):
                    rm = gp.tile([128, TP], F32, name=f"rpT{m}", tag=f"rp{m}")
                    for n in range(2):
                        ps = pp.tile([128, 512], F32, name="ps_rp", tag="psa")
                        for k in range(2):
                            nc.tensor.matmul(
                                ps[:], rk[k][:, m * 128:(m + 1) * 128],
                                hT[k][:, n * 512:n * 512 + 512],
                                start=(k == 0), stop=(k == 1))
                        nc.scalar.activation(rm[:, n * 512:(n + 1) * 512],
                                             ps[:], AFT.Identity, bias=b1[m][:])
                    rpT.append(rm)

            def gate_in(m, name):
                g = gp.tile([128, TP], F32, name=name, tag="gin", bufs=2)
                if it == 0:
                    nc.vector.tensor_scalar(g[:], xpT[m][:], b1[m][:], None,
                                            OP.add)
                else:
                    nc.vector.tensor_tensor(g[:], xpT[m][:], rpT[m][:], OP.add)
                return g

            zT, rT, cT = [], [], []
            for k in range(2):
                zin = gate_in(0 + k, f"zin{k}")
                z = gp.tile([128, TP], F32, name=f"zT{k}", tag=f"zT{k}")
                nc.scalar.activation(z[:], zin[:], AFT.Sigmoid)
                zT.append(z)
            for k in range(2):
                rin = gate_in(2 + k, f"rin{k}")
                r = gp.tile([128, TP], F32, name=f"rT{k}", tag=f"rT{k}")
                nc.scalar.activation(r[:], rin[:], AFT.Sigmoid)
                rT.append(r)
            for k in range(2):
                rrh = gp.tile([128, TP], F32, name=f"rrh{k}", tag="gtmp", bufs=2)
                if it == 0:
                    nc.vector.tensor_scalar(rrh[:], rT[k][:], b1[4 + k][:],
                                            None, OP.mult)
                else:
                    nc.vector.tensor_tensor(rrh[:], rT[k][:], rpT[4 + k][:],
                                            OP.mult)
                hin = gp.tile([128, TP], F32, name=f"hin{k}", tag="gin", bufs=2)
                nc.vector.tensor_tensor(hin[:], rrh[:], xpT[4 + k][:], OP.add)
                hh = gp.tile([128, TP], F32, name=f"hh{k}", tag=f"hh{k}")
                nc.scalar.activation(hh[:], hin[:], AFT.Tanh)
                zh = gp.tile([128, TP], F32, name=f"zh{k}", tag="gtmp", bufs=2)
                nc.vector.tensor_tensor(zh[:], zT[k][:], hh[:], OP.mult)
                cc = gp.tile([128, TP], F32, name=f"cT{k}", tag=f"cT{k}")
                nc.vector.tensor_tensor(cc[:], hh[:], zh[:], OP.subtract)
                cT.append(cc)
            for k in range(2):
                nc.vector.tensor_tensor_scan(hT[k][:, 1:TP + 1], zT[k][:],
                                             cT[k][:], 0.0, OP.mult, OP.add)
        self.tap("hT0", hT[0][:])

        # ---- post MLP ----
        x2T = []
        for m in range(2):
            xm = ap_.tile([128, TP], F32R, name=f"x2T{m}")
            for n in range(2):
                ps = pp.tile([128, 512], F32, name="ps_x2", tag="psa")
                for k in range(2):
                    nc.tensor.matmul(ps[:], pw1[k][:, m * 128:(m + 1) * 128],
                                     hT[k][:, 1 + n * 512:1 + n * 512 + 512],
                                     start=(k == 0), stop=(k == 1))
                nc.scalar.activation(xm[:, n * 512:(n + 1) * 512], ps[:],
                                     AFT.Relu, bias=pb1[m][:])
            x2T.append(xm)
        x3T = ap_.tile([128, TP], F32R, name="x3T")
        for n in range(2):
            ps = pp.tile([128, 512], F32, name="ps_x3", tag="psa")
            for k in range(2):
                nc.tensor.matmul(ps[:], pw2[k][:],
                                 x2T[k][:, n * 512:(n + 1) * 512],
                                 start=(k == 0), stop=(k == 1))
            nc.scalar.activation(x3T[:, n * 512:(n + 1) * 512], ps[:],
                                 AFT.Relu, bias=pb2[:])
        headsT = []
        for m in range(2):
            hm = ap_.tile([128, TP], F32, name=f"headsT{m}")
            for n in range(2):
                ps = pp.tile([128, 512], F32, name="ps_hd", tag="psa")
                nc.tensor.matmul(ps[:], hW[:, m * 128:(m + 1) * 128],
                                 x3T[:, n * 512:(n + 1) * 512],
                                 start=True, stop=True)
                nc.scalar.activation(hm[:, n * 512:(n + 1) * 512], ps[:],
                                     AFT.Identity, bias=hB[m][:])
            headsT.append(hm)
        self.tap("headsT0", headsT[0][:])

        # ---- transpose to frame-major + control math (function-batched) ----
        cp = ctx.enter_context(tc.tile_pool(name="ctlF", bufs=1))
        hf, eh, sg, am, mg, negm, erec, f0cs = [], [], [], [], [], [], [], []
        for c in range(NCH):
            h_ = cp.tile([128, 256], F32, name=f"headsF{c}")
            for g in range(2):
                ps = pp.tile([128, 128], F32, name="ps_tr", tag="pst")
                nc.tensor.transpose(ps[:], headsT[g][:, c * 128:(c + 1) * 128],
                                    eye[:])
                nc.scalar.activation(h_[:, g * 128:(g + 1) * 128], ps[:],
                                     AFT.Copy)
            hf.append(h_)
        for c in range(NCH):
            hmax = cp.tile([128, 1], F32, name=f"hmax{c}")
            nc.vector.tensor_reduce(hmax[:], hf[c][:, 1:65], AX, OP.max)
            ng = cp.tile([128, 1], F32, name=f"negm{c}")
            nc.vector.tensor_scalar(ng[:], hmax[:], -1.0, None, OP.mult)
            negm.append(ng)
        for c in range(NCH):
            e_ = cp.tile([128, 64], F32, name=f"eh{c}")
            nc.scalar.activation(e_[:], hf[c][:, 1:65], AFT.Exp, bias=negm[c][:])
            eh.append(e_)
        for c in range(NCH):
            esum = cp.tile([128, 1], F32, name=f"esum{c}")
            nc.vector.tensor_reduce(esum[:], eh[c][:], AX, OP.add)
            er = cp.tile([128, 1], F32, name=f"erec{c}")
            nc.vector.reciprocal(er[:], esum[:])
            erec.append(er)
            sg_ = cp.tile([128, 64], F32, name=f"sg{c}")
            nc.vector.tensor_scalar(sg_[:], eh[c][:], er[:], None, OP.mult)
            sg.append(sg_)
            if c == 0:
                self.tap("harm_dist0", sg_[:])
            am_ = cp.tile([128, 1], F32, name=f"am{c}")
            am.append(am_)
            mg_ = cp.tile([128, NB], F32, name=f"magsF{c}")
            mg.append(mg_)
        # exp_sigmoid chains, batched by activation function
        for c in range(NCH):
            nc.scalar.activation(sg[c][:], sg[c][:], AFT.Sigmoid)
            nc.scalar.activation(am[c][:], hf[c][:, 0:1], AFT.Sigmoid)
            nc.scalar.activation(mg[c][:], hf[c][:, 65:130], AFT.Sigmoid)
        for c in range(NCH):
            nc.scalar.activation(sg[c][:], sg[c][:], AFT.Ln)
            nc.scalar.activation(am[c][:], am[c][:], AFT.Ln)
            nc.scalar.activation(mg[c][:], mg[c][:], AFT.Ln)
        for c in range(NCH):
            nc.scalar.activation(sg[c][:], sg[c][:], AFT.Exp, scale=LN10,
                                 bias=ln2col[:])
            nc.scalar.activation(am[c][:], am[c][:], AFT.Exp, scale=LN10,
                                 bias=ln2col[:])
            nc.scalar.activation(mg[c][:], mg[c][:], AFT.Exp, scale=LN10,
                                 bias=ln2col[:])
        for c in range(NCH):
            nc.scalar.activation(am[c][:], am[c][:], AFT.Sigmoid)
            nc.scalar.activation(mg[c][:], mg[c][:], AFT.Sigmoid, bias=m5col[:])
        for c in range(NCH):
            nc.scalar.activation(am[c][:], am[c][:], AFT.Ln)
            nc.scalar.activation(mg[c][:], mg[c][:], AFT.Ln)
        for c in range(NCH):
            nc.scalar.activation(am[c][:], am[c][:], AFT.Exp, scale=LN10,
                                 bias=ln2col[:])
            nc.scalar.activation(mg[c][:], mg[c][:], AFT.Exp, scale=LN10,
                                 bias=ln2col[:])
        # nyquist mask, normalize, a = amp*hd; mags out
        for c in range(NCH):
            f0c = cp.tile([128, 1], F32, name=f"f0c{c}")
            nc.sync.dma_start(f0c[:], din["f0pad"].ap()
                              [c * 128:(c + 1) * 128].unsqueeze(1))
            msk = cp.tile([128, 64], F32, name=f"mskA{c}")
            nc.vector.tensor_scalar(msk[:], c8000oh[:], f0c[:], None, OP.is_gt)
            nc.vector.tensor_tensor(sg[c][:], sg[c][:], msk[:], OP.mult)
            den = cp.tile([128, 1], F32, name=f"den{c}")
            nc.vector.tensor_reduce(den[:], sg[c][:], AX, OP.add)
            dre = cp.tile([128, 1], F32, name=f"dre{c}")
            nc.vector.reciprocal(dre[:], den[:])
            nc.vector.tensor_scalar(sg[c][:], sg[c][:], dre[:], None, OP.mult)
            aF = cp.tile([128, 64], F32, name=f"aF{c}")
            nc.vector.tensor_scalar(aF[:], sg[c][:], am[c][:], None, OP.mult)
            pat = pp.tile([64, 128], F32, name="ps_at", tag="pst")
            nc.tensor.transpose(pat[:], aF[:], eye[:])
            atx = cp.tile([64, 128], F16, name=f"atx{c}")
            nc.scalar.activation(atx[:], pat[:], AFT.Copy)
            nc.sync.dma_start(self.aT.ap()[0:64, c * 128:(c + 1) * 128],
                              atx[:])
            if c == NCH - 1:
                # frame-999 interp fix: aT[:,1000] <- aT[:,999] (col 103 here)
                nc.sync.dma_start(self.aT.ap()[0:64, 1000:1001],
                                  atx[:, 103:104])
            nc.sync.dma_start(self.scr["mags_dram"].ap()
                              [c * 128:(c + 1) * 128], mg[c][:])
            if c == 0:
                self.tap("aF0", aF[:])
                self.tap("mags0", mg[c][:])
        # zero aT padding cols (uninit dram would poison the PE reduce)
        zat = cp.tile([64, 8], F16, name="zat")
        nc.vector.memset(zat[:], 0.0)
        nc.sync.dma_start(self.aT.ap()[0:64, TP:TP + 8], zat[:])


    # -------------------------------------------------------------------
    def _phase_b(self, tc, ctx):
        """Harmonic synthesis -> audio_dram[0:NSP] (harmonic part)."""
        nc = self.nc
        din = self.din
        wp = ctx.enter_context(tc.tile_pool(name="wpB", bufs=1))
        fp_ = ctx.enter_context(tc.tile_pool(name="frameB", bufs=1))
        from contextlib import ExitStack
        sp = ctx.enter_context(tc.tile_pool(name="smallB", bufs=2))
        hctx = ExitStack()

        def cload(name, rows, cols):
            tl = wp.tile([rows, cols], F32, name=name)
            nc.sync.dma_start(tl[:], din["c_" + name].ap()[0:rows])
            return tl

        w0j = cload("w0j", 128, 64)
        w1j = cload("w1j", 128, 64)
        rst = sp.tile([128, 8], F32, name="rst", tag="rst")
        nc.sync.dma_start(rst[:], din["c_redbd"].ap()[0:128])
        red16 = wp.tile([128, 8], F16, name="red16")
        nc.vector.tensor_copy(red16[:], rst[:])
        hst = sp.tile([6, 128], F32, name="hst", tag="rst")
        nc.sync.dma_start(hst[:], din["c_hbd6"].ap()[0:6])
        hbd6 = wp.tile([6, 128], F16, name="hbd6")
        nc.vector.tensor_copy(hbd6[:], hst[:])
        mpicol = wp.tile([128, 1], F32, name="mpicol")
        nc.vector.memset(mpicol[:], -float(np.float32(np.pi)))

        # ---- per-chunk dphi / intra-frame scan (batched over chunks) ----
        w0ts = cload("w0ts", 128, 64)
        w1ts = cload("w1ts", 128, 64)
        ones = cload("ones", 128, TP)
        F0C = fp_.tile([128, 8], F32, name="F0C")
        F0N = fp_.tile([128, 8], F32, name="F0N")
        nc.sync.dma_start(F0C[:], din["f0pad"].ap()[0:TP]
                          .rearrange("(c p) -> p c", p=128))
        nc.sync.dma_start(F0N[:], din["f0pad"].ap()[1:TP + 1]
                          .rearrange("(c p) -> p c", p=128))
        dpA = fp_.tile([128, 512], F32, name="dpA")
        dp3 = dpA[:].rearrange("p (c j) -> p c j", j=64)
        tA = fp_.tile([128, 512], F32, name="tA")
        w0b = w0ts[:].unsqueeze(1).broadcast_to([128, 8, 64])
        w1b = w1ts[:].unsqueeze(1).broadcast_to([128, 8, 64])
        nc.vector.tensor_tensor(tA[:].rearrange("p (c j) -> p c j", j=64),
                                w1b, F0N[:].unsqueeze(2)
                                .broadcast_to([128, 8, 64]), OP.mult)
        nc.vector.tensor_tensor(dp3, w0b, F0C[:].unsqueeze(2)
                                .broadcast_to([128, 8, 64]), OP.mult)
        nc.vector.tensor_tensor(dpA[:], dpA[:], tA[:], OP.add)
        pcA = fp_.tile([128, 512], F32, name="pcA")
        for c in range(NCH):
            nc.vector.tensor_tensor_scan(pcA[:, c * 64:(c + 1) * 64],
                                         ones[:, 0:64],
                                         dpA[:, c * 64:(c + 1) * 64],
                                         0.0, OP.mult, OP.add)
        nc.sync.dma_start(self.scr["delta_dram"].ap()
                          .rearrange("(c p) -> p c", p=128).unsqueeze(2),
                          pcA[:].rearrange("p (c j) -> p c j", j=64)
                          [:, :, 63:64])

        # ---- frame-level compensated phase scan ----
        drow = fp_.tile([1, TP], F32, name="drow")
        nc.sync.dma_start(drow[:], self.scr["delta_dram"].ap()[0:TP]
                          .unsqueeze(0))
        zc1 = fp_.tile([1, 1], F32, name="zc1")
        nc.vector.memset(zc1[:], 0.0)
        hi = fp_.tile([1, TP + 1], F32, name="hi")
        nc.vector.tensor_copy(hi[:, 0:1], zc1[:])
        nc.vector.tensor_tensor_scan(hi[:, 1:TP + 1], ones[0:1, :], drow[:],
                                     0.0, OP.mult, OP.add)
        dd = fp_.tile([1, TP], F32, name="dd")
        nc.vector.tensor_tensor(dd[:], hi[:, 1:TP + 1], hi[:, 0:TP],
                                OP.subtract)
        ee = fp_.tile([1, TP], F32, name="ee")
        nc.vector.tensor_tensor(ee[:], drow[:], dd[:], OP.subtract)
        lo = fp_.tile([1, TP + 1], F32, name="lo")
        nc.vector.tensor_copy(lo[:, 0:1], zc1[:])
        nc.vector.tensor_tensor_scan(lo[:, 1:TP + 1], ones[0:1, :], ee[:],
                                     0.0, OP.mult, OP.add)
        kff = fp_.tile([1, TP], F32, name="kff")
        nc.vector.tensor_scalar(kff[:], hi[:, 0:TP], INV2PI, MAGIC, OP.mult,
                                OP.add)
        nc.vector.tensor_scalar(kff[:], kff[:], MAGIC, None, OP.subtract)
        cwf = fp_.tile([1, TP], F32, name="cwf")
        nc.vector.cody_waite_cascade(cwf[:], hi[:, 0:TP], kff[:], *CW_FRAME)
        phr = fp_.tile([1, TP], F32, name="phr")
        nc.vector.tensor_tensor(phr[:], cwf[:], lo[:, 0:TP], OP.add)
        nc.sync.dma_start(self.scr["phired_dram"].ap()[0:TP].unsqueeze(0),
                          phr[:])

        # ---- batched per-sample phase rows -> 3 bf16 components ----
        PHIcol = fp_.tile([128, 8], F32, name="PHIcol")
        nc.sync.dma_start(PHIcol[:], self.scr["phired_dram"].ap()
                          .rearrange("(c p) -> p c", p=128))
        phFA = fp_.tile([128, 512], F32, name="phFA")
        nc.vector.tensor_tensor(phFA[:].rearrange("p (c j) -> p c j", j=64),
                                pcA[:].rearrange("p (c j) -> p c j", j=64),
                                PHIcol[:].unsqueeze(2)
                                .broadcast_to([128, 8, 64]), OP.add)
        res = phFA
        for i in range(3):
            a16 = fp_.tile([128, 512], F16, name=f"cmp{i}")
            nc.vector.tensor_copy(a16[:], res[:])
            nc.sync.dma_start(self.phiC[i].ap()
                              .rearrange("(c p j) -> p c j", p=128, j=64),
                              a16[:].rearrange("p (c j) -> p c j", j=64))
            if i < 2:
                r2 = fp_.tile([128, 512], F32, name=f"resA{i}")
                nc.vector.tensor_tensor(r2[:], res[:], a16[:], OP.subtract)
                res = r2

        # ---- h-major oscillator bank over chunk pairs ----
        # partitions = [chunk_even harmonics 1..64 | chunk_odd harmonics]
        # th = h (.) phi via PE outer product; mask = (h (.) dphi) < pi;
        # amp-weighted reduce over h via PE with block-diag ones.
        hp = hctx.enter_context(tc.tile_pool(name="oscB", bufs=1))
        ppt = hctx.enter_context(tc.tile_pool(name="psTh", bufs=2,
                                              space="PSUM"))
        ppr = hctx.enter_context(tc.tile_pool(name="psR", bufs=2,
                                              space="PSUM"))
        RH2 = self.scr["rh_dram"].ap().rearrange("(r cs) -> r cs",
                                                 cs=NCH * 8192)
        AT = self.aT.ap()
        PI_F = float(np.float32(np.pi))
        for q in range(NCH // 2):
            c0 = 2 * q
            phip = hp.tile([6, 8193], F16, name="phip", tag="phip", bufs=2)
            amC = hp.tile([128, 128], F16, name="amC", tag="amC", bufs=2)
            amN = hp.tile([128, 128], F16, name="amN", tag="amN", bufs=2)
            for r in range(2):
                cc = c0 + r
                for i in range(3):
                    PX = self.phiC[i].ap()
                    row = 3 * r + i
                    if cc > 0:
                        nc.sync.dma_start(
                            phip[row:row + 1, :],
                            PX[cc * 8192 - 1:(cc + 1) * 8192].unsqueeze(0))
                    else:
                        nc.sync.dma_start(
                            phip[row:row + 1, 1:8193],
                            PX[0:8192].unsqueeze(0))
                        nc.sync.dma_start(phip[row:row + 1, 0:1],
                                          PX[0:1].unsqueeze(0))
                nc.sync.dma_start(amC[64 * r:64 * r + 64, :],
                                  AT[0:64, cc * 128:(cc + 1) * 128])
                nc.sync.dma_start(amN[64 * r:64 * r + 64, :],
                                  AT[0:64, cc * 128 + 1:(cc + 1) * 128 + 1])
            for t in range(8):
                s0 = t * 1024
                f0_ = t * 16
                thp = ppt.tile([128, 1025], F32, name="thp", tag="thp")
                for dl0, dl1 in ((0, 512), (512, 1024), (1024, 1025)):
                    nc.tensor.matmul(thp[:, dl0:dl1], hbd6[:],
                                     phip[:, s0 + dl0:s0 + dl1],
                                     start=True, stop=True)
                ths = hp.tile([128, 1025], F32, name="ths", tag="ths", bufs=3)
                nc.vector.tensor_copy(ths[:], thp[:])
                msk = hp.tile([128, 1024], F16, name="msk", tag="msk", bufs=3)
                nc.vector.scalar_tensor_tensor(msk[:], ths[:, 1:1025], -PI_F,
                                               ths[:, 0:1024],
                                               OP.add, OP.is_lt)
                kfm = hp.tile([128, 1024], F32, name="kfm", tag="kfm", bufs=3)
                nc.scalar.activation(kfm[:], ths[:, 1:1025], AFT.Copy,
                                     scale=INV2PI, bias=MAGIC)
                kf = hp.tile([128, 1024], F32, name="kf", tag="kf", bufs=3)
                nc.scalar.activation(kf[:], kfm[:], AFT.Copy, bias=-MAGIC)
                u = hp.tile([128, 1024], F32, name="u", tag="u", bufs=3)
                nc.vector.scalar_tensor_tensor(u[:], kf[:], -TWOPI_F32,
                                               ths[:, 1:1025],
                                               OP.mult, OP.add)
                s16 = hp.tile([128, 1024], F16, name="s16", tag="s16", bufs=3)
                nc.scalar.activation(s16[:], u[:], AFT.Sin)
                s16m = hp.tile([128, 1024], F16, name="s16m", tag="s16m",
                               bufs=3)
                nc.gpsimd.tensor_tensor(s16m[:], s16[:], msk[:], OP.mult)
                mC = hp.tile([128, 1024], F16, name="mC", tag="mC", bufs=3)
                mN = hp.tile([128, 1024], F16, name="mN", tag="mN", bufs=3)
                s3 = s16m[:].rearrange("p (f j) -> p f j", j=64)
                aCb = amC[:, f0_:f0_ + 16].unsqueeze(2).broadcast_to(
                    [128, 16, 64])
                aNb = amN[:, f0_:f0_ + 16].unsqueeze(2).broadcast_to(
                    [128, 16, 64])
                nc.vector.tensor_tensor(
                    mC[:].rearrange("p (f j) -> p f j", j=64), s3, aCb,
                    OP.mult)
                nc.gpsimd.tensor_tensor(
                    mN[:].rearrange("p (f j) -> p f j", j=64), s3, aNb,
                    OP.mult)
                rE = hp.tile([4, 1024], F32, name="rE", tag="rE", bufs=2)
                for kk in range(2):
                    dl = slice(kk * 512, (kk + 1) * 512)
                    rps = ppr.tile([4, 512], F32, name="rps", tag="rps")
                    nc.tensor.matmul(rps[:], red16[:, 0:4], mC[:, dl],
                                     start=True, stop=False)
                    nc.tensor.matmul(rps[:], red16[:, 4:8], mN[:, dl],
                                     start=False, stop=True)
                    nc.scalar.activation(rE[:, dl], rps[:], AFT.Copy)
                for rr in range(2):
                    cc = c0 + rr
                    nc.sync.dma_start(
                        RH2[0:2, cc * 8192 + s0:cc * 8192 + s0 + 1024],
                        rE[2 * rr:2 * rr + 2, :])

        hctx.close()

        # ---- noise path: per-frame FIR in frequency domain ----
        eye = cload("eye", 128, 128)
        gre = cload("g_re", 65, 129)
        gim = cload("g_im", 65, 129)
        dre = cload("d_re", 64, 129)
        dim_ = cload("d_im", 64, 129)
        di_re0 = cload("di_re", 128, 256)
        di_im0 = cload("di_im", 128, 256)
        di_re1 = wp.tile([1, 256], F32, name="di_re1")
        nc.sync.dma_start(di_re1[:], din["c_di_re"].ap()[128:129])
        di_im1 = wp.tile([1, 256], F32, name="di_im1")
        nc.sync.dma_start(di_im1[:], din["c_di_im"].ap()[128:129])
        rnd = {}
        for nm, src in [("gre", gre), ("gim", gim), ("dre", dre),
                        ("dim", dim_), ("dire0", di_re0), ("diim0", di_im0),
                        ("dire1", di_re1), ("diim1", di_im1)]:
            dst = wp.tile(list(src.shape), F32R, name=nm + "_r")
            nc.vector.tensor_copy(dst[:], src[:])
            rnd[nm] = dst

        np_ = ctx.enter_context(tc.tile_pool(name="noiseB", bufs=2))
        pp = ctx.enter_context(tc.tile_pool(name="psB", bufs=2, space="PSUM"))
        zt = wp.tile([128, 256], F32, name="zt")
        nc.vector.memset(zt[:], 0.0)
        nc.sync.dma_start(self.scr["of_dram"].ap()[0:4], zt[0:4, :])
        nc.sync.dma_start(self.scr["of_dram"].ap()[4 + TP:8 + TP], zt[0:4, :])

        for sc_ in range(4):
            magsT = np_.tile([128, 256], F32R, name="magsT", tag="magsT")
            frT = np_.tile([128, 256], F32R, name="frT", tag="frT")
            for half in range(2):
                c = sc_ * 2 + half
                mstage = np_.tile([128, 128], F32, name="mstage", tag="mstage")
                nc.vector.memset(mstage[:], 0.0)
                nc.sync.dma_start(mstage[:, 0:NB], self.scr["mags_dram"].ap()
                                  [c * 128:(c + 1) * 128])
                pst = pp.tile([128, 128], F32, name="pst", tag="pst")
                nc.tensor.transpose(pst[:], mstage[:], eye[:])
                nc.vector.tensor_copy(magsT[:, half * 128:(half + 1) * 128],
                                      pst[:])
                fstage = np_.tile([128, 128], F32, name="fstage", tag="fstage")
                nc.vector.memset(fstage[:], 0.0)
                nc.sync.dma_start(
                    fstage[:, 0:64], din["noisepad"].ap()
                    [c * 8192:(c + 1) * 8192].rearrange("(p j) -> p j", j=64))
                pst2 = pp.tile([128, 128], F32, name="pst2", tag="pst")
                nc.tensor.transpose(pst2[:], fstage[:], eye[:])
                nc.vector.tensor_copy(frT[:, half * 128:(half + 1) * 128],
                                      pst2[:])
            # spectra [129, 256] as (128-row tile, 1-row tile) pairs
            def spectrum(nm, lhsT, kdim, rhs):
                big = np_.tile([128, 256], F32, name=nm + "b", tag=nm + "b")
                ps0 = pp.tile([128, 256], F32, name="ps_sp", tag="pss")
                nc.tensor.matmul(ps0[:], lhsT[0:kdim, 0:128], rhs[0:kdim, :],
                                 start=True, stop=True)
                nc.vector.tensor_copy(big[:], ps0[:])
                one = np_.tile([1, 256], F32, name=nm + "o", tag=nm + "o")
                ps1 = pp.tile([1, 256], F32, name="ps_sp1", tag="pss1")
                nc.tensor.matmul(ps1[:], lhsT[0:kdim, 128:129], rhs[0:kdim, :],
                                 start=True, stop=True)
                nc.vector.tensor_copy(one[:], ps1[:])
                return big, one
            irre = spectrum("irre", rnd["gre"], 65, magsT)
            irim = spectrum("irim", rnd["gim"], 65, magsT)
            frre = spectrum("frre", rnd["dre"], 64, frT)
            frim = spectrum("frim", rnd["dim"], 64, frT)
            # P = FR * IR (complex), f32r
            pres, pims = [], []
            for idx in range(2):
                fre, fim = frre[idx], frim[idx]
                ire, iim = irre[idx], irim[idx]
                rows = fre.shape[0]
                t1 = np_.tile([rows, 256], F32, name="t1", tag=f"t1_{idx}")
                t2_ = np_.tile([rows, 256], F32, name="t2n", tag=f"t2n_{idx}")
                pre = np_.tile([rows, 256], F32R, name="pre", tag=f"pre_{idx}")
                pim = np_.tile([rows, 256], F32R, name="pim", tag=f"pim_{idx}")
                nc.vector.tensor_tensor(t1[:], fre[:], ire[:], OP.mult)
                nc.vector.tensor_tensor(t2_[:], fim[:], iim[:], OP.mult)
                nc.vector.tensor_tensor(pre[:], t1[:], t2_[:], OP.subtract)
                nc.vector.tensor_tensor(t1[:], fre[:], iim[:], OP.mult)
                nc.vector.tensor_tensor(t2_[:], fim[:], ire[:], OP.mult)
                nc.vector.tensor_tensor(pim[:], t1[:], t2_[:], OP.add)
                pres.append(pre)
                pims.append(pim)
            # outfullT [256u (2 chunks), 256t] then transpose to of_dram rows
            for mu in range(2):
                pso = pp.tile([128, 256], F32, name="ps_of", tag="pss")
                nc.tensor.matmul(pso[:], rnd["dire0"][:, mu * 128:(mu + 1) * 128],
                                 pres[0][:], start=True, stop=False)
                nc.tensor.matmul(pso[:], rnd["dire1"][:, mu * 128:(mu + 1) * 128],
                                 pres[1][:], start=False, stop=False)
                nc.tensor.matmul(pso[:], rnd["diim0"][:, mu * 128:(mu + 1) * 128],
                                 pims[0][:], start=False, stop=False)
                nc.tensor.matmul(pso[:], rnd["diim1"][:, mu * 128:(mu + 1) * 128],
                                 pims[1][:], start=False, stop=True)
                ofT = np_.tile([128, 256], F32, name="ofT", tag="ofT")
                nc.vector.tensor_copy(ofT[:], pso[:])
                for th_ in range(2):
                    psb = pp.tile([128, 128], F32, name="ps_tb", tag="pst")
                    nc.tensor.transpose(psb[:], ofT[:, th_ * 128:(th_ + 1) * 128],
                                        eye[:])
                    ofb = np_.tile([128, 128], F32, name="ofb", tag="ofb")
                    nc.vector.tensor_copy(ofb[:], psb[:])
                    row0 = 4 + sc_ * 256 + th_ * 128
                    nc.sync.dma_start(
                        self.scr["of_dram"].ap()
                        [row0:row0 + 128, mu * 128:(mu + 1) * 128], ofb[:])

        # ---- overlap-add + crop + combine with harmonic ----
        OF = self.scr["of_dram"].ap()
        for c in range(NCH):
            r0 = 4 + c * 128
            acc = np_.tile([128, 64], F32, name="acc", tag="acc")
            ta = np_.tile([128, 64], F32, name="ta", tag="ta")
            nc.sync.dma_start(acc[:], OF[r0:r0 + 128, 62:126])
            nc.sync.dma_start(ta[:], OF[r0 - 1:r0 + 127, 126:190])
            nc.vector.tensor_tensor(acc[:], acc[:], ta[:], OP.add)
            ta2 = np_.tile([128, 64], F32, name="ta2", tag="ta")
            nc.sync.dma_start(ta2[:], OF[r0 - 2:r0 + 126, 190:254])
            nc.vector.tensor_tensor(acc[:], acc[:], ta2[:], OP.add)
            ta3 = np_.tile([128, 2], F32, name="ta3", tag="tas")
            nc.sync.dma_start(ta3[:], OF[r0 - 3:r0 + 125, 254:256])
            nc.vector.tensor_tensor(acc[:, 0:2], acc[:, 0:2], ta3[:], OP.add)
            ta4 = np_.tile([128, 62], F32, name="ta4", tag="tas")
            nc.sync.dma_start(ta4[:], OF[r0 + 1:r0 + 129, 0:62])
            nc.vector.tensor_tensor(acc[:, 2:64], acc[:, 2:64], ta4[:], OP.add)
            RH = self.scr["rh_dram"].ap()
            r0F = np_.tile([128, 64], F32, name="r0F", tag="r0F")
            r1F = np_.tile([128, 64], F32, name="r1F", tag="r1F")
            nc.sync.dma_start(r0F[:], RH[c * 8192:(c + 1) * 8192]
                              .rearrange("(p j) -> p j", j=64))
            nc.sync.dma_start(r1F[:], RH[NCH * 8192 + c * 8192:
                                         NCH * 8192 + (c + 1) * 8192]
                              .rearrange("(p j) -> p j", j=64))
            hw0 = np_.tile([128, 64], F32, name="hw0", tag="hw", bufs=2)
            nc.vector.tensor_tensor(hw0[:], r0F[:], w0j[:], OP.mult)
            nc.vector.tensor_tensor(acc[:], acc[:], hw0[:], OP.add)
            hw1 = np_.tile([128, 64], F32, name="hw1", tag="hw", bufs=2)
            nc.vector.tensor_tensor(hw1[:], r1F[:], w1j[:], OP.mult)
            nc.vector.tensor_tensor(acc[:], acc[:], hw1[:], OP.add)
            nc.sync.dma_start(self.scr["audio_dram"].ap()
                              [c * 8192:(c + 1) * 8192]


# revision 19
# speedup vs baseline: 1.0690x; 1.0690x over previous
"""Trainium2 Bass kernel for the DDSP decoder (nn_DDSPDecoder).

Sharding: pure data-parallel over batch B=8 across 8 NeuronCores; control-net
weights and reverb IR replicated; each core synthesizes one example.

Per-core pipeline (frames padded 1000->1024; GRU solved by quasi-linear Picard
iterations whose linear part is the hardware scan; harmonic phase factored
through the fundamental with a compensated two-float frame scan + Cody-Waite
range reduction; filtered noise done in the frequency domain with constant
DFT matrices; reverb as a 131072-pt 4-step FFT convolution on the PE).
"""
import os
import sys

if "/opt/trn_rl_repo" not in sys.path:
    sys.path.insert(0, "/opt/trn_rl_repo")

import numpy as np

import concourse.bass as bass  # noqa: F401
import concourse.tile as tile
from concourse import bacc, bass_utils, mybir
from concourse.alu_op_type import AluOpType as OP

F32 = mybir.dt.float32
F32R = mybir.dt.float32r
F16 = mybir.dt.bfloat16
I32 = mybir.dt.int32
AFT = mybir.ActivationFunctionType

SR = 16000
T = 1000
TP = 1024
NCH = 8                      # frame chunks of 128
NS = 64000
NSP = 65536
H = 64
NB = 65
N_CORES = 8
GRU_ITERS = 5
LN10 = float(np.float32(np.log(np.float32(10.0))))
LN2 = float(np.float32(np.log(2.0)))
TWO_PI = 2.0 * np.pi
TWOPI_F32 = float(np.float32(TWO_PI))
INV2PI = float(np.float32(1.0 / TWO_PI))
AX = mybir.AxisListType.X
MAGIC = 12582912.0  # 1.5 * 2**23: float32 round-to-nearest-int trick

DEBUG_TAPS = bool(int(os.environ.get("KERNEL_DEBUG_TAPS", "0")))


# ---------------------------------------------------------------------------
# host-side constants
# ---------------------------------------------------------------------------

def _cw_chunks(kmax_bits):
    nb = 24 - kmax_bits
    x = TWO_PI
    cs = []
    for _ in range(2):
        m, e = np.frexp(x)
        q = float(np.float32(np.ldexp(np.round(np.ldexp(m, nb)), e - nb)))
        cs.append(q)
        x -= q
    cs.append(float(np.float32(x)))
    return cs

CW_FRAME = _cw_chunks(12)
CW_SAMP = _cw_chunks(9)


def _noise_matrices():
    n = 128
    t = np.arange(n)
    C = np.zeros((NB, n))
    C[0] = 1.0 / n
    for m in range(1, 64):
        C[m] = 2.0 / n * np.cos(2 * np.pi * m * t / n)
    C[64] = 1.0 / n * np.cos(np.pi * t)
    w = 0.5 - 0.5 * np.cos(2.0 * np.pi * t / n)
    Cw = C * np.fft.fftshift(w)[None, :]
    Cc = np.zeros_like(Cw)
    Cc[:, (t + n // 2) % n] = Cw
    F = np.exp(-2j * np.pi * np.outer(t, np.arange(129)) / 256.0)
    Gc = Cc @ F
    Ff = np.exp(-2j * np.pi * np.outer(np.arange(64), np.arange(129)) / 256.0)
    tt = np.arange(256)
    DIre = np.zeros((129, 256))
    DIim = np.zeros((129, 256))
    for f in range(129):
        mult = 1.0 if f in (0, 128) else 2.0
        DIre[f] = mult * np.cos(2 * np.pi * f * tt / 256.0) / 256.0
        DIim[f] = -mult * np.sin(2 * np.pi * f * tt / 256.0) / 256.0
    return Gc, Ff, DIre, DIim


def _make_consts():
    f32 = np.float32
    c = {}
    j = np.arange(64, dtype=np.float64)
    sc = TWO_PI / SR
    c["w0ts"] = np.tile(f32((1.0 - j / 64.0) * sc), (128, 1))
    c["w1ts"] = np.tile(f32((j / 64.0) * sc), (128, 1))
    c["w0j"] = np.tile(f32(1.0 - j / 64.0), (128, 1))
    c["w1j"] = np.tile(f32(j / 64.0), (128, 1))
    h = np.arange(1, H + 1, dtype=np.float64)
    c["hrow"] = np.tile(f32(h), (128, 1))
    c["pioh"] = np.tile(f32(np.pi / h), (128, 1))
    c["c8000oh"] = np.tile(f32(8000.0 / h), (128, 1))
    c["eye"] = np.eye(128, dtype=np.float32)
    c["ones"] = np.ones((128, TP), np.float32)
    Gc, Ff, DIre, DIim = _noise_matrices()
    c["g_re"], c["g_im"] = f32(Gc.real), f32(Gc.imag)          # [65,129]
    c["d_re"], c["d_im"] = f32(Ff.real), f32(Ff.imag)          # [64,129]
    c["di_re"], c["di_im"] = f32(DIre), f32(DIim)              # [129,256]
    n1 = np.arange(256)
    k1 = np.arange(256)
    n2 = np.arange(512)
    W256 = np.exp(-2j * np.pi * np.outer(n1, k1) / 256.0)
    W512 = np.exp(-2j * np.pi * np.outer(n2, n2) / 512.0)
    TWf = np.exp(-2j * np.pi * np.outer(k1, n2) / 131072.0)
    TWi = np.exp(2j * np.pi * np.outer(n2, k1) / 131072.0)
    isc = 0.25 / 131072.0
    c["w256c"], c["w256s"] = f32(W256.real), f32(W256.imag)
    c["w512c"], c["w512s"] = f32(W512.real), f32(W512.imag)
    c["tw2c"] = f32(np.cos(2 * np.pi * n2 / 512.0)).reshape(512, 1)
    c["tw2s"] = f32(np.sin(2 * np.pi * n2 / 512.0)).reshape(512, 1)
    c["twfc"], c["twfs"] = f32(TWf.real), f32(TWf.imag)        # [256,512]
    c["twic"], c["twis"] = f32(TWi.real), f32(TWi.imag)        # [512,256]
    c["i2c"] = f32(np.cos(2 * np.pi * np.outer(k1, n1) / 256.0) * isc)   # [k1,n1]
    c["i2s"] = f32(-np.sin(2 * np.pi * np.outer(k1, n1) / 256.0) * isc)
    hbd = np.zeros((6, 128))
    for comp in range(3):
        hbd[comp, 0:64] = np.arange(1, 65)
        hbd[3 + comp, 64:128] = np.arange(1, 65)
    c["hbd6"] = f32(hbd)
    hb4 = np.zeros((4, 128))
    hb4[0, 0:64] = np.arange(1, 65)
    hb4[1, 0:64] = np.arange(1, 65)
    hb4[2, 64:128] = np.arange(1, 65)
    hb4[3, 64:128] = np.arange(1, 65)
    c["hbd4"] = f32(hb4)
    red = np.zeros((128, 8))
    red[0:64, 0] = 1.0    # mC lower strip -> r0 of chunk c0
    red[64:128, 2] = 1.0  # mC upper strip -> r0 of chunk c1
    red[0:64, 5] = 1.0    # mN lower strip -> r1 of chunk c0
    red[64:128, 7] = 1.0  # mN upper strip -> r1 of chunk c1
    c["redbd"] = f32(red)
    return {k: np.ascontiguousarray(v, np.float32) for k, v in c.items()}

CONSTS = _make_consts()


def _prep_weights(I):
    f32 = lambda v: np.ascontiguousarray(v, np.float32)
    d = {}
    d["pre_w"] = f32(I["pre_w"])                       # [2,128]
    d["pre_b"] = f32(np.reshape(I["pre_b"], (128, 1)))
    d["gru_k"] = f32(I["gru_k"])                       # [128,768]
    d["gru_rk"] = f32(I["gru_rk"])                     # [256,768]
    d["b0col"] = f32(np.reshape(I["gru_b"][0], (768, 1)))
    d["b1col"] = f32(np.reshape(I["gru_b"][1], (768, 1)))
    # z/r gates take b0+b1 folded into the input-projection bias; the h gate
    # keeps b1 inside the reset product (keras reset_after)
    b01 = I["gru_b"][0] + I["gru_b"][1]
    b01[512:768] = I["gru_b"][0][512:768]
    d["b01col"] = f32(np.reshape(b01, (768, 1)))
    d["post_w1"] = f32(I["post_w1"])
    d["pb1col"] = f32(np.reshape(I["post_b1"], (256, 1)))
    d["post_w2"] = f32(I["post_w2"])
    d["pb2col"] = f32(np.reshape(I["post_b2"], (128, 1)))
    hw = np.zeros((128, 256), np.float32)
    hw[:, 0:1] = I["amp_w"]
    hw[:, 1:65] = I["harm_w"]
    hw[:, 65:130] = I["noise_w"]
    d["headsW"] = hw
    hb = np.zeros((256, 1), np.float32)
    hb[0, 0] = np.asarray(I["amp_b"]).reshape(-1)[0]
    hb[1:65, 0] = I["harm_b"]
    hb[65:130, 0] = I["noise_b"]
    d["headsB"] = hb
    return d

W_SHAPES = {
    "pre_w": (2, 128), "pre_b": (128, 1), "gru_k": (128, 768),
    "gru_rk": (256, 768), "b0col": (768, 1), "b1col": (768, 1),
    "b01col": (768, 1),
    "post_w1": (256, 256), "pb1col": (256, 1), "post_w2": (256, 128),
    "pb2col": (128, 1), "headsW": (128, 256), "headsB": (256, 1),
}


# ---------------------------------------------------------------------------
# program builder
# ---------------------------------------------------------------------------

class Prog:
    def __init__(self, debug_taps=False):
        self.debug = debug_taps
        self.taps = {}
        nc = bacc.Bacc("TRN2", num_devices=N_CORES, target_bir_lowering=False,
                       debug=False)
        self.nc = nc
        self.din = {}
        for name, shape in [("f0pad", [TP + 32]), ("loudpad", [TP + 32]),
                            ("noisepad", [NSP]), ("rirpad", [NSP])]:
            self.din[name] = nc.dram_tensor(name, shape, F32, kind="ExternalInput")
        for k, s in W_SHAPES.items():
            self.din["w_" + k] = nc.dram_tensor("w_" + k, list(s), F32,
                                                kind="ExternalInput")
        for k, v in CONSTS.items():
            self.din["c_" + k] = nc.dram_tensor("c_" + k, list(v.shape), F32,
                                                kind="ExternalInput")
        self.out = nc.dram_tensor("audio_out", [NS], F32, kind="ExternalOutput")
        self.scr = {}
        for name, shape in [("a_dram", [TP + 8, 64]), ("mags_dram", [TP, NB]),
                            ("delta_dram", [TP]),
                            ("phired_dram", [TP]), ("audio_dram", [NSP]),
                            ("rh_dram", [2 * NCH * 8192]),
                            ("sq_re", [512, 256]), ("sq_im", [512, 256]),
                            ("of_dram", [TP + 8, 256])]:
            self.scr[name] = nc.dram_tensor(name, shape, F32)
        self.scr["dphic_dram"] = nc.dram_tensor("dphic_dram", [2, TP * 64],
                                                F16)
        self.aT = nc.dram_tensor("aT_dram", [64, TP + 8], F16)
        self.phiC = [nc.dram_tensor(f"phi{i}_dram", [TP * 64], F16)
                     for i in range(3)]

    def tap(self, name, ap):
        if not self.debug:
            return
        t = self.nc.dram_tensor("tap_" + name, list(ap.shape), ap.dtype,
                                kind="ExternalOutput")
        self.nc.sync.dma_start(t.ap(), ap)
        self.taps[name] = t

    def build(self):
        nc = self.nc
        with tile.TileContext(nc) as tc:
            self._body(tc)
        nc.compile()
        return nc

    # -------------------------------------------------------------------
    def _body(self, tc):
        from contextlib import ExitStack
        with ExitStack() as ctx:
            self._phase_a(tc, ctx)
        with ExitStack() as ctx:
            self._phase_b(tc, ctx)
        with ExitStack() as ctx:
            self._phase_c(tc, ctx)

    # -------------------------------------------------------------------
    def _phase_a(self, tc, ctx):
        nc = self.nc
        din = self.din
        wp = ctx.enter_context(tc.tile_pool(name="wp", bufs=1))
        ap_ = ctx.enter_context(tc.tile_pool(name="actv", bufs=1))
        tp = ctx.enter_context(tc.tile_pool(name="tmpA", bufs=2))
        pp = ctx.enter_context(tc.tile_pool(name="psA", bufs=4, space="PSUM"))

        def wload(name, rows, cols, dtype=F32, rnd=False):
            """Load weight/const dram tensor into <=128-row SBUF tiles."""
            tiles = []
            nchunk = (rows + 127) // 128
            for i in range(nchunk):
                r0, r1 = i * 128, min((i + 1) * 128, rows)
                tl = wp.tile([r1 - r0, cols], dtype, name=f"{name}_{i}")
                if rnd:
                    st = tp.tile([r1 - r0, cols], F32, name=f"{name}_st{i}",
                                 tag="wstage")
                    nc.sync.dma_start(st[:], din[name].ap()[r0:r1])
                    nc.vector.tensor_copy(tl[:], st[:])
                else:
                    nc.sync.dma_start(tl[:], din[name].ap()[r0:r1])
                tiles.append(tl)
            return tiles

        prew_s0 = tp.tile([1, 128], F32, name="prew_s0")
        prew_s1 = tp.tile([1, 128], F32, name="prew_s1")
        nc.sync.dma_start(prew_s0[:], din["w_pre_w"].ap()[0:1])
        nc.sync.dma_start(prew_s1[:], din["w_pre_w"].ap()[1:2])
        prew0 = wp.tile([1, 128], F32R, name="prew0")
        prew1 = wp.tile([1, 128], F32R, name="prew1")
        nc.vector.tensor_copy(prew0[:], prew_s0[:])
        nc.vector.tensor_copy(prew1[:], prew_s1[:])
        pre_b = wload("w_pre_b", 128, 1)[0]
        gru_k = wload("w_gru_k", 128, 768, F32R, rnd=True)[0]
        rk = wload("w_gru_rk", 256, 768, F32R, rnd=True)
        b1 = wload("w_b1col", 768, 1)
        b01 = wload("w_b01col", 768, 1)
        pw1 = wload("w_post_w1", 256, 256, F32R, rnd=True)
        pb1 = wload("w_pb1col", 256, 1)
        pw2 = wload("w_post_w2", 256, 128, F32R, rnd=True)
        pb2 = wload("w_pb2col", 128, 1)[0]
        hW = wload("w_headsW", 128, 256, F32R, rnd=True)[0]
        hB = wload("w_headsB", 256, 1)
        eye = wload("c_eye", 128, 128)[0]
        c8000oh = wload("c_c8000oh", 128, 64)[0]

        ln2col = wp.tile([128, 1], F32, name="ln2col")
        nc.vector.memset(ln2col[:], LN2)
        m5col = wp.tile([128, 1], F32, name="m5col")
        nc.vector.memset(m5col[:], -5.0)

        # ---- f0m / loud rows [1, TP] ----
        f0row = tp.tile([1, TP], F32, name="f0row")
        loudrow = tp.tile([1, TP], F32, name="loudrow")
        nc.sync.dma_start(f0row[:], din["f0pad"].ap()[0:TP].unsqueeze(0))
        nc.sync.dma_start(loudrow[:], din["loudpad"].ap()[0:TP].unsqueeze(0))
        lnf0 = tp.tile([1, TP], F32, name="lnf0")
        nc.scalar.activation(lnf0[:], f0row[:], AFT.Ln)
        a_log = 12.0 / float(np.log(2.0))
        c_log = 69.0 - 12.0 * float(np.log2(440.0))
        f0mR = ap_.tile([1, TP], F32R, name="f0mR")
        loudR = ap_.tile([1, TP], F32R, name="loudR")
        nc.scalar.activation(f0mR[:], lnf0[:], AFT.Copy, scale=a_log,
                             bias=c_log)
        nc.scalar.activation(loudR[:], loudrow[:], AFT.Copy)

        # ---- x1T [128, TP] ----
        x1T = ap_.tile([128, TP], F32R, name="x1T")
        for n in range(2):
            ps = pp.tile([128, 512], F32, name="ps_x1", tag="psa")
            nc.tensor.matmul(ps[:], prew0[:], f0mR[:, n * 512:(n + 1) * 512],
                             start=True, stop=False)
            nc.tensor.matmul(ps[:], prew1[:], loudR[:, n * 512:(n + 1) * 512],
                             start=False, stop=True)
            nc.scalar.activation(x1T[:, n * 512:(n + 1) * 512], ps[:],
                                 AFT.Relu, bias=pre_b[:])
        self.tap("x1T", x1T[:])

        # ---- xpT 6 x [128, TP] ----
        xpT = []
        for m in range(6):
            xm = ap_.tile([128, TP], F32, name=f"xpT{m}")
            for n in range(2):
                ps = pp.tile([128, 512], F32, name="ps_xp", tag="psa")
                nc.tensor.matmul(ps[:], gru_k[:, m * 128:(m + 1) * 128],
                                 x1T[:, n * 512:(n + 1) * 512],
                                 start=True, stop=True)
                nc.scalar.activation(xm[:, n * 512:(n + 1) * 512], ps[:],
                                     AFT.Identity, bias=b01[m][:])
            xpT.append(xm)

        # ---- GRU Picard ----
        zcol = wp.tile([128, 1], F32, name="zcol")
        nc.vector.memset(zcol[:], 0.0)
        hT = []
        for k in range(2):
            ht = ap_.tile([128, TP + 1], F32R, name=f"hT{k}")
            nc.vector.tensor_copy(ht[:, 0:1], zcol[:])
            hT.append(ht)

        gp = ctx.enter_context(tc.tile_pool(name="gru", bufs=1))
        for it in range(GRU_ITERS):
            rpT = []
            if it > 0:
                for m in range(6):
                    rm = gp.tile([128, TP], F32, name=f"rpT{m}", tag=f"rp{m}")
                    for n in range(2):
                        ps = pp.tile([128, 512], F32, name="ps_rp", tag="psa")
                        for k in range(2):
                            nc.tensor.matmul(
                                ps[:], rk[k][:, m * 128:(m + 1) * 128],
                                hT[k][:, n * 512:n * 512 + 512],
                                start=(k == 0), stop=(k == 1))
                        nc.scalar.activation(rm[:, n * 512:(n + 1) * 512],
                                             ps[:], AFT.Identity, bias=b1[m][:])
                    rpT.append(rm)

            def gate_in(m, name):
                g = gp.tile([128, TP], F32, name=name, tag="gin", bufs=2)
                if it == 0:
                    nc.vector.tensor_scalar(g[:], xpT[m][:], b1[m][:], None,
                                            OP.add)
                else:
                    nc.vector.tensor_tensor(g[:], xpT[m][:], rpT[m][:], OP.add)
                return g

            zT, rT, cT = [], [], []
            for k in range(2):
                zin = gate_in(0 + k, f"zin{k}")
                z = gp.tile([128, TP], F32, name=f"zT{k}", tag=f"zT{k}")
                nc.scalar.activation(z[:], zin[:], AFT.Sigmoid)
                zT.append(z)
            for k in range(2):
                rin = gate_in(2 + k, f"rin{k}")
                r = gp.tile([128, TP], F32, name=f"rT{k}", tag=f"rT{k}")
                nc.scalar.activation(r[:], rin[:], AFT.Sigmoid)
                rT.append(r)
            for k in range(2):
                rrh = gp.tile([128, TP], F32, name=f"rrh{k}", tag="gtmp", bufs=2)
                if it == 0:
                    nc.vector.tensor_scalar(rrh[:], rT[k][:], b1[4 + k][:],
                                            None, OP.mult)
                else:
                    nc.vector.tensor_tensor(rrh[:], rT[k][:], rpT[4 + k][:],
                                            OP.mult)
                hin = gp.tile([128, TP], F32, name=f"hin{k}", tag="gin", bufs=2)
                nc.vector.tensor_tensor(hin[:], rrh[:], xpT[4 + k][:], OP.add)
                hh = gp.tile([128, TP], F32, name=f"hh{k}", tag=f"hh{k}")
                nc.scalar.activation(hh[:], hin[:], AFT.Tanh)
                zh = gp.tile([128, TP], F32, name=f"zh{k}", tag="gtmp", bufs=2)
                nc.vector.tensor_tensor(zh[:], zT[k][:], hh[:], OP.mult)
                cc = gp.tile([128, TP], F32, name=f"cT{k}", tag=f"cT{k}")
                nc.vector.tensor_tensor(cc[:], hh[:], zh[:], OP.subtract)
                cT.append(cc)
            for k in range(2):
                nc.vector.tensor_tensor_scan(hT[k][:, 1:TP + 1], zT[k][:],
                                             cT[k][:], 0.0, OP.mult, OP.add)
        self.tap("hT0", hT[0][:])

        # ---- post MLP ----
        x2T = []
        for m in range(2):
            xm = ap_.tile([128, TP], F32R, name=f"x2T{m}")
            for n in range(2):
                ps = pp.tile([128, 512], F32, name="ps_x2", tag="psa")
                for k in range(2):
                    nc.tensor.matmul(ps[:], pw1[k][:, m * 128:(m + 1) * 128],
                                     hT[k][:, 1 + n * 512:1 + n * 512 + 512],
                                     start=(k == 0), stop=(k == 1))
                nc.scalar.activation(xm[:, n * 512:(n + 1) * 512], ps[:],
                                     AFT.Relu, bias=pb1[m][:])
            x2T.append(xm)
        x3T = ap_.tile([128, TP], F32R, name="x3T")
        for n in range(2):
            ps = pp.tile([128, 512], F32, name="ps_x3", tag="psa")
            for k in range(2):
                nc.tensor.matmul(ps[:], pw2[k][:],
                                 x2T[k][:, n * 512:(n + 1) * 512],
                                 start=(k == 0), stop=(k == 1))
            nc.scalar.activation(x3T[:, n * 512:(n + 1) * 512], ps[:],
                                 AFT.Relu, bias=pb2[:])
        headsT = []
        for m in range(2):
            hm = ap_.tile([128, TP], F32, name=f"headsT{m}")
            for n in range(2):
                ps = pp.tile([128, 512], F32, name="ps_hd", tag="psa")
                nc.tensor.matmul(ps[:], hW[:, m * 128:(m + 1) * 128],
                                 x3T[:, n * 512:(n + 1) * 512],
                                 start=True, stop=True)
                nc.scalar.activation(hm[:, n * 512:(n + 1) * 512], ps[:],
                                     AFT.Identity, bias=hB[m][:])
            headsT.append(hm)
        self.tap("headsT0", headsT[0][:])

        # ---- transpose to frame-major + control math (function-batched) ----
        cp = ctx.enter_context(tc.tile_pool(name="ctlF", bufs=1))
        hf, eh, sg, am, mg, negm, erec, f0cs = [], [], [], [], [], [], [], []
        for c in range(NCH):
            h_ = cp.tile([128, 256], F32, name=f"headsF{c}")
            for g in range(2):
                ps = pp.tile([128, 128], F32, name="ps_tr", tag="pst")
                nc.tensor.transpose(ps[:], headsT[g][:, c * 128:(c + 1) * 128],
                                    eye[:])
                nc.scalar.activation(h_[:, g * 128:(g + 1) * 128], ps[:],
                                     AFT.Copy)
            hf.append(h_)
        for c in range(NCH):
            hmax = cp.tile([128, 1], F32, name=f"hmax{c}")
            nc.vector.tensor_reduce(hmax[:], hf[c][:, 1:65], AX, OP.max)
            ng = cp.tile([128, 1], F32, name=f"negm{c}")
            nc.vector.tensor_scalar(ng[:], hmax[:], -1.0, None, OP.mult)
            negm.append(ng)
        for c in range(NCH):
            e_ = cp.tile([128, 64], F32, name=f"eh{c}")
            nc.scalar.activation(e_[:], hf[c][:, 1:65], AFT.Exp, bias=negm[c][:])
            eh.append(e_)
        for c in range(NCH):
            esum = cp.tile([128, 1], F32, name=f"esum{c}")
            nc.vector.tensor_reduce(esum[:], eh[c][:], AX, OP.add)
            er = cp.tile([128, 1], F32, name=f"erec{c}")
            nc.vector.reciprocal(er[:], esum[:])
            erec.append(er)
            sg_ = cp.tile([128, 64], F32, name=f"sg{c}")
            nc.vector.tensor_scalar(sg_[:], eh[c][:], er[:], None, OP.mult)
            sg.append(sg_)
            if c == 0:
                self.tap("harm_dist0", sg_[:])
            am_ = cp.tile([128, 1], F32, name=f"am{c}")
            am.append(am_)
            mg_ = cp.tile([128, NB], F32, name=f"magsF{c}")
            mg.append(mg_)
        # exp_sigmoid chains, batched by activation function
        for c in range(NCH):
            nc.scalar.activation(sg[c][:], sg[c][:], AFT.Sigmoid)
            nc.scalar.activation(am[c][:], hf[c][:, 0:1], AFT.Sigmoid)
            nc.scalar.activation(mg[c][:], hf[c][:, 65:130], AFT.Sigmoid)
        for c in range(NCH):
            nc.scalar.activation(sg[c][:], sg[c][:], AFT.Ln)
            nc.scalar.activation(am[c][:], am[c][:], AFT.Ln)
            nc.scalar.activation(mg[c][:], mg[c][:], AFT.Ln)
        for c in range(NCH):
            nc.scalar.activation(sg[c][:], sg[c][:], AFT.Exp, scale=LN10,
                                 bias=ln2col[:])
            nc.scalar.activation(am[c][:], am[c][:], AFT.Exp, scale=LN10,
                                 bias=ln2col[:])
            nc.scalar.activation(mg[c][:], mg[c][:], AFT.Exp, scale=LN10,
                                 bias=ln2col[:])
        for c in range(NCH):
            nc.scalar.activation(am[c][:], am[c][:], AFT.Sigmoid)
            nc.scalar.activation(mg[c][:], mg[c][:], AFT.Sigmoid, bias=m5col[:])
        for c in range(NCH):
            nc.scalar.activation(am[c][:], am[c][:], AFT.Ln)
            nc.scalar.activation(mg[c][:], mg[c][:], AFT.Ln)
        for c in range(NCH):
            nc.scalar.activation(am[c][:], am[c][:], AFT.Exp, scale=LN10,
                                 bias=ln2col[:])
            nc.scalar.activation(mg[c][:], mg[c][:], AFT.Exp, scale=LN10,
                                 bias=ln2col[:])
        # nyquist mask, normalize, a = amp*hd; mags out
        for c in range(NCH):
            f0c = cp.tile([128, 1], F32, name=f"f0c{c}")
            nc.sync.dma_start(f0c[:], din["f0pad"].ap()
                              [c * 128:(c + 1) * 128].unsqueeze(1))
            msk = cp.tile([128, 64], F32, name=f"mskA{c}")
            nc.vector.tensor_scalar(msk[:], c8000oh[:], f0c[:], None, OP.is_gt)
            nc.vector.tensor_tensor(sg[c][:], sg[c][:], msk[:], OP.mult)
            den = cp.tile([128, 1], F32, name=f"den{c}")
            nc.vector.tensor_reduce(den[:], sg[c][:], AX, OP.add)
            dre = cp.tile([128, 1], F32, name=f"dre{c}")
            nc.vector.reciprocal(dre[:], den[:])
            nc.vector.tensor_scalar(sg[c][:], sg[c][:], dre[:], None, OP.mult)
            aF = cp.tile([128, 64], F32, name=f"aF{c}")
            nc.vector.tensor_scalar(aF[:], sg[c][:], am[c][:], None, OP.mult)
            pat = pp.tile([64, 128], F32, name="ps_at", tag="pst")
            nc.tensor.transpose(pat[:], aF[:], eye[:])
            atx = cp.tile([64, 128], F16, name=f"atx{c}")
            nc.scalar.activation(atx[:], pat[:], AFT.Copy)
            nc.sync.dma_start(self.aT.ap()[0:64, c * 128:(c + 1) * 128],
                              atx[:])
            if c == NCH - 1:
                # frame-999 interp fix: aT[:,1000] <- aT[:,999] (col 103 here)
                nc.sync.dma_start(self.aT.ap()[0:64, 1000:1001],
                                  atx[:, 103:104])
            nc.sync.dma_start(self.scr["mags_dram"].ap()
                              [c * 128:(c + 1) * 128], mg[c][:])
            if c == 0:
                self.tap("aF0", aF[:])
                self.tap("mags0", mg[c][:])
        # zero aT padding cols (uninit dram would poison the PE reduce)
        zat = cp.tile([64, 8], F16, name="zat")
        nc.vector.memset(zat[:], 0.0)
        nc.sync.dma_start(self.aT.ap()[0:64, TP:TP + 8], zat[:])


    # -------------------------------------------------------------------
    def _phase_b(self, tc, ctx):
        """Harmonic synthesis -> audio_dram[0:NSP] (harmonic part)."""
        nc = self.nc
        din = self.din
        wp = ctx.enter_context(tc.tile_pool(name="wpB", bufs=1))
        fp_ = ctx.enter_context(tc.tile_pool(name="frameB", bufs=1))
        from contextlib import ExitStack
        sp = ctx.enter_context(tc.tile_pool(name="smallB", bufs=2))
        hctx = ExitStack()

        def cload(name, rows, cols):
            tl = wp.tile([rows, cols], F32, name=name)
            nc.sync.dma_start(tl[:], din["c_" + name].ap()[0:rows])
            return tl

        w0j = cload("w0j", 128, 64)
        w1j = cload("w1j", 128, 64)
        rst = sp.tile([128, 8], F32, name="rst", tag="rst")
        nc.sync.dma_start(rst[:], din["c_redbd"].ap()[0:128])
        red16 = wp.tile([128, 8], F16, name="red16")
        nc.vector.tensor_copy(red16[:], rst[:])
        hst = sp.tile([6, 128], F32, name="hst", tag="rst")
        nc.sync.dma_start(hst[:], din["c_hbd6"].ap()[0:6])
        hbd6 = wp.tile([6, 128], F16, name="hbd6")
        nc.vector.tensor_copy(hbd6[:], hst[:])
        mpicol = wp.tile([128, 1], F32, name="mpicol")
        nc.vector.memset(mpicol[:], -float(np.float32(np.pi)))

        # ---- per-chunk dphi / intra-frame scan (batched over chunks) ----
        w0ts = cload("w0ts", 128, 64)
        w1ts = cload("w1ts", 128, 64)
        ones = cload("ones", 128, TP)
        F0C = fp_.tile([128, 8], F32, name="F0C")
        F0N = fp_.tile([128, 8], F32, name="F0N")
        nc.sync.dma_start(F0C[:], din["f0pad"].ap()[0:TP]
                          .rearrange("(c p) -> p c", p=128))
        nc.sync.dma_start(F0N[:], din["f0pad"].ap()[1:TP + 1]
                          .rearrange("(c p) -> p c", p=128))
        dpA = fp_.tile([128, 512], F32, name="dpA")
        dp3 = dpA[:].rearrange("p (c j) -> p c j", j=64)
        tA = fp_.tile([128, 512], F32, name="tA")
        w0b = w0ts[:].unsqueeze(1).broadcast_to([128, 8, 64])
        w1b = w1ts[:].unsqueeze(1).broadcast_to([128, 8, 64])
        nc.vector.tensor_tensor(tA[:].rearrange("p (c j) -> p c j", j=64),
                                w1b, F0N[:].unsqueeze(2)
                                .broadcast_to([128, 8, 64]), OP.mult)
        nc.vector.tensor_tensor(dp3, w0b, F0C[:].unsqueeze(2)
                                .broadcast_to([128, 8, 64]), OP.mult)
        nc.vector.tensor_tensor(dpA[:], dpA[:], tA[:], OP.add)
        # per-sample fundamental dphi rows, compensated bf16 hi+lo, for the
        # osc-bank nyquist mask (thd = h (.) dphi via f16 matmul)
        dhi = fp_.tile([128, 512], F16, name="dhi")
        nc.vector.tensor_copy(dhi[:], dpA[:])
        dlo32 = fp_.tile([128, 512], F32, name="dlo32")
        nc.vector.tensor_tensor(dlo32[:], dpA[:], dhi[:], OP.subtract)
        dlo = fp_.tile([128, 512], F16, name="dlo")
        nc.vector.tensor_copy(dlo[:], dlo32[:])
        DPHC = self.scr["dphic_dram"].ap()
        nc.sync.dma_start(DPHC[0:1].rearrange("r (c p j) -> p (r c) j",
                                              p=128, j=64),
                          dhi[:].rearrange("p (c j) -> p c j", j=64))
        nc.sync.dma_start(DPHC[1:2].rearrange("r (c p j) -> p (r c) j",
                                              p=128, j=64),
                          dlo[:].rearrange("p (c j) -> p c j", j=64))
        pcA = fp_.tile([128, 512], F32, name="pcA")
        for c in range(NCH):
            nc.vector.tensor_tensor_scan(pcA[:, c * 64:(c + 1) * 64],
                                         ones[:, 0:64],
                                         dpA[:, c * 64:(c + 1) * 64],
                                         0.0, OP.mult, OP.add)
        nc.sync.dma_start(self.scr["delta_dram"].ap()
                          .rearrange("(c p) -> p c", p=128).unsqueeze(2),
                          pcA[:].rearrange("p (c j) -> p c j", j=64)
                          [:, :, 63:64])

        # ---- frame-level compensated phase scan ----
        drow = fp_.tile([1, TP], F32, name="drow")
        nc.sync.dma_start(drow[:], self.scr["delta_dram"].ap()[0:TP]
                          .unsqueeze(0))
        zc1 = fp_.tile([1, 1], F32, name="zc1")
        nc.vector.memset(zc1[:], 0.0)
        hi = fp_.tile([1, TP + 1], F32, name="hi")
        nc.vector.tensor_copy(hi[:, 0:1], zc1[:])
        nc.vector.tensor_tensor_scan(hi[:, 1:TP + 1], ones[0:1, :], drow[:],
                                     0.0, OP.mult, OP.add)
        dd = fp_.tile([1, TP], F32, name="dd")
        nc.vector.tensor_tensor(dd[:], hi[:, 1:TP + 1], hi[:, 0:TP],
                                OP.subtract)
        ee = fp_.tile([1, TP], F32, name="ee")
        nc.vector.tensor_tensor(ee[:], drow[:], dd[:], OP.subtract)
        lo = fp_.tile([1, TP + 1], F32, name="lo")
        nc.vector.tensor_copy(lo[:, 0:1], zc1[:])
        nc.vector.tensor_tensor_scan(lo[:, 1:TP + 1], ones[0:1, :], ee[:],
                                     0.0, OP.mult, OP.add)
        kff = fp_.tile([1, TP], F32, name="kff")
        nc.vector.tensor_scalar(kff[:], hi[:, 0:TP], INV2PI, MAGIC, OP.mult,
                                OP.add)
        nc.vector.tensor_scalar(kff[:], kff[:], MAGIC, None, OP.subtract)
        cwf = fp_.tile([1, TP], F32, name="cwf")
        nc.vector.cody_waite_cascade(cwf[:], hi[:, 0:TP], kff[:], *CW_FRAME)
        phr = fp_.tile([1, TP], F32, name="phr")
        nc.vector.tensor_tensor(phr[:], cwf[:], lo[:, 0:TP], OP.add)
        nc.sync.dma_start(self.scr["phired_dram"].ap()[0:TP].unsqueeze(0),
                          phr[:])

        # ---- batched per-sample phase rows -> 3 bf16 components ----
        PHIcol = fp_.tile([128, 8], F32, name="PHIcol")
        nc.sync.dma_start(PHIcol[:], self.scr["phired_dram"].ap()
                          .rearrange("(c p) -> p c", p=128))
        phFA = fp_.tile([128, 512], F32, name="phFA")
        nc.vector.tensor_tensor(phFA[:].rearrange("p (c j) -> p c j", j=64),
                                pcA[:].rearrange("p (c j) -> p c j", j=64),
                                PHIcol[:].unsqueeze(2)
                                .broadcast_to([128, 8, 64]), OP.add)
        res = phFA
        for i in range(3):
            a16 = fp_.tile([128, 512], F16, name=f"cmp{i}")
            nc.vector.tensor_copy(a16[:], res[:])
            nc.sync.dma_start(self.phiC[i].ap()
                              .rearrange("(c p j) -> p c j", p=128, j=64),
                              a16[:].rearrange("p (c j) -> p c j", j=64))
            if i < 2:
                r2 = fp_.tile([128, 512], F32, name=f"resA{i}")
                nc.vector.tensor_tensor(r2[:], res[:], a16[:], OP.subtract)
                res = r2

        # ---- h-major oscillator bank over chunk pairs ----
        # partitions = [chunk_even harmonics 1..64 | chunk_odd harmonics]
        # th = h (.) phi via PE outer product; mask = (h (.) dphi) < pi with
        # dphi via a contraction-2 matmul from per-sample dphi rows;
        # amp-weighted reduce over h via PE with block-diag ones.
        hb4st = sp.tile([4, 128], F32, name="hb4st", tag="rst")
        nc.sync.dma_start(hb4st[:], din["c_hbd4"].ap()[0:4])
        hbd4 = wp.tile([4, 128], F16, name="hbd4")
        nc.vector.tensor_copy(hbd4[:], hb4st[:])
        hp = hctx.enter_context(tc.tile_pool(name="oscB", bufs=1))
        ppt = hctx.enter_context(tc.tile_pool(name="psTh", bufs=2,
                                              space="PSUM"))
        ppd = hctx.enter_context(tc.tile_pool(name="psTd", bufs=1,
                                              space="PSUM"))
        ppr = hctx.enter_context(tc.tile_pool(name="psR", bufs=2,
                                              space="PSUM"))
        RH2 = self.scr["rh_dram"].ap().rearrange("(r cs) -> r cs",
                                                 cs=NCH * 8192)
        AT = self.aT.ap()
        PI_F = float(np.float32(np.pi))
        for q in range(NCH // 2):
            c0 = 2 * q
            phip = hp.tile([6, 8193], F16, name="phip", tag="phip", bufs=2)
            amC = hp.tile([128, 128], F16, name="amC", tag="amC", bufs=2)
            amN = hp.tile([128, 128], F16, name="amN", tag="amN", bufs=2)
            for r in range(2):
                cc = c0 + r
                for i in range(3):
                    PX = self.phiC[i].ap()
                    row = 3 * r + i
                    if cc > 0:
                        nc.sync.dma_start(
                            phip[row:row + 1, :],
                            PX[cc * 8192 - 1:(cc + 1) * 8192].unsqueeze(0))
                    else:
                        nc.sync.dma_start(
                            phip[row:row + 1, 1:8193],
                            PX[0:8192].unsqueeze(0))
                        nc.sync.dma_start(phip[row:row + 1, 0:1],
                                          PX[0:1].unsqueeze(0))
                nc.sync.dma_start(amC[64 * r:64 * r + 64, :],
                                  AT[0:64, cc * 128:(cc + 1) * 128])
                nc.sync.dma_start(amN[64 * r:64 * r + 64, :],
                                  AT[0:64, cc * 128 + 1:(cc + 1) * 128 + 1])
            for t in range(8):
                s0 = t * 1024
                f0_ = t * 16
                if t % 4 == 0:
                    rEacc = hp.tile([4, 4096], F32, name="rEacc",
                                    tag="rEacc", bufs=2)
                a0 = s0 % 4096
                dphi4 = hp.tile([4, 1024], F16, name="dphi4", tag="dphi4",
                                bufs=2)
                for r in range(2):
                    cc = c0 + r
                    nc.sync.dma_start(
                        dphi4[2 * r:2 * r + 2, :],
                        DPHC[0:2, cc * 8192 + s0:cc * 8192 + s0 + 1024])
                thp = ppt.tile([128, 1024], F32, name="thp", tag="thp")
                thd = ppd.tile([128, 1024], F32, name="thd", tag="thd")
                for dl0, dl1 in ((0, 512), (512, 1024)):
                    nc.tensor.matmul(thp[:, dl0:dl1], hbd6[:],
                                     phip[:, s0 + 1 + dl0:s0 + 1 + dl1],
                                     start=True, stop=True)
                    nc.tensor.matmul(thd[:, dl0:dl1], hbd4[:],
                                     dphi4[:, dl0:dl1],
                                     start=True, stop=True)
                kfm = hp.tile([128, 1024], F32, name="kfm", tag="kfm", bufs=2)
                nc.scalar.activation(kfm[:], thp[:], AFT.Copy,
                                     scale=INV2PI, bias=MAGIC)
                ku = hp.tile([128, 1024], F32, name="ku", tag="ku", bufs=2)
                nc.vector.tensor_scalar(ku[:], kfm[:], -MAGIC, -TWOPI_F32,
                                        OP.add, OP.mult)
                u = hp.tile([128, 1024], F32, name="u", tag="u", bufs=2)
                nc.vector.tensor_tensor(u[:], ku[:], thp[:], OP.add)
                msk = hp.tile([128, 1024], F16, name="msk", tag="msk", bufs=3)
                nc.vector.tensor_scalar(msk[:], thd[:], PI_F, None, OP.is_lt)
                s16 = hp.tile([128, 1024], F16, name="s16", tag="s16", bufs=3)
                nc.scalar.activation(s16[:], u[:], AFT.Sin)
                s16m = hp.tile([128, 1024], F16, name="s16m", tag="s16m",
                               bufs=3)
                nc.vector.tensor_tensor(s16m[:], s16[:], msk[:], OP.mult)
                mC = hp.tile([128, 1024], F16, name="mC", tag="mC", bufs=3)
                mN = hp.tile([128, 1024], F16, name="mN", tag="mN", bufs=3)
                s3 = s16m[:].rearrange("p (f j) -> p f j", j=64)
                aCb = amC[:, f0_:f0_ + 16].unsqueeze(2).broadcast_to(
                    [128, 16, 64])
                aNb = amN[:, f0_:f0_ + 16].unsqueeze(2).broadcast_to(
                    [128, 16, 64])
                nc.gpsimd.tensor_tensor(
                    mC[:].rearrange("p (f j) -> p f j", j=64), s3, aCb,
                    OP.mult)
                nc.gpsimd.tensor_tensor(
                    mN[:].rearrange("p (f j) -> p f j", j=64), s3, aNb,
                    OP.mult)
                for kk in range(2):
                    dl = slice(kk * 512, (kk + 1) * 512)
                    rps = ppr.tile([4, 512], F32, name="rps", tag="rps")
                    nc.tensor.matmul(rps[:], red16[:, 0:4], mC[:, dl],
                                     start=True, stop=False)
                    nc.tensor.matmul(rps[:], red16[:, 4:8], mN[:, dl],
                                     start=False, stop=True)
                    nc.scalar.activation(rEacc[:, a0 + kk * 512:
                                               a0 + (kk + 1) * 512],
                                         rps[:], AFT.Copy)
                if t % 4 == 3:
                    h0 = s0 + 1024 - 4096
                    for rr in range(2):
                        cc = c0 + rr
                        nc.sync.dma_start(
                            RH2[0:2, cc * 8192 + h0:cc * 8192 + h0 + 4096],
                            rEacc[2 * rr:2 * rr + 2, :])

        hctx.close()

        # ---- noise path: per-frame FIR in frequency domain ----
        eye = cload("eye", 128, 128)
        gre = cload("g_re", 65, 129)
        gim = cload("g_im", 65, 129)
        dre = cload("d_re", 64, 129)
        dim_ = cload("d_im", 64, 129)
        di_re0 = cload("di_re", 128, 256)
        di_im0 = cload("di_im", 128, 256)
        di_re1 = wp.tile([1, 256], F32, name="di_re1")
        nc.sync.dma_start(di_re1[:], din["c_di_re"].ap()[128:129])
        di_im1 = wp.tile([1, 256], F32, name="di_im1")
        nc.sync.dma_start(di_im1[:], din["c_di_im"].ap()[128:129])
        rnd = {}
        for nm, src in [("gre", gre), ("gim", gim), ("dre", dre),
                        ("dim", dim_), ("dire0", di_re0), ("diim0", di_im0),
                        ("dire1", di_re1), ("diim1", di_im1)]:
            dst = wp.tile(list(src.shape), F32R, name=nm + "_r")
            nc.vector.tensor_copy(dst[:], src[:])
            rnd[nm] = dst

        np_ = ctx.enter_context(tc.tile_pool(name="noiseB", bufs=2))
        pp = ctx.enter_context(tc.tile_pool(name="psB", bufs=2, space="PSUM"))
        zt = wp.tile([128, 256], F32, name="zt")
        nc.vector.memset(zt[:], 0.0)
        nc.sync.dma_start(self.scr["of_dram"].ap()[0:4], zt[0:4, :])
        nc.sync.dma_start(self.scr["of_dram"].ap()[4 + TP:8 + TP], zt[0:4, :])

        for sc_ in range(4):
            magsT = np_.tile([128, 256], F32R, name="magsT", tag="magsT")
            frT = np_.tile([128, 256], F32R, name="frT", tag="frT")
            for half in range(2):
                c = sc_ * 2 + half
                mstage = np_.tile([128, 128], F32, name="mstage", tag="mstage")
                nc.vector.memset(mstage[:], 0.0)
                nc.sync.dma_start(mstage[:, 0:NB], self.scr["mags_dram"].ap()
                                  [c * 128:(c + 1) * 128])
                pst = pp.tile([128, 128], F32, name="pst", tag="pst")
                nc.tensor.transpose(pst[:], mstage[:], eye[:])
                nc.vector.tensor_copy(magsT[:, half * 128:(half + 1) * 128],
                                      pst[:])
                fstage = np_.tile([128, 128], F32, name="fstage", tag="fstage")
                nc.vector.memset(fstage[:], 0.0)
                nc.sync.dma_start(
                    fstage[:, 0:64], din["noisepad"].ap()
                    [c * 8192:(c + 1) * 8192].rearrange("(p j) -> p j", j=64))
                pst2 = pp.tile([128, 128], F32, name="pst2", tag="pst")
                nc.tensor.transpose(pst2[:], fstage[:], eye[:])
                nc.vector.tensor_copy(frT[:, half * 128:(half + 1) * 128],
                                      pst2[:])
            # spectra [129, 256] as (128-row tile, 1-row tile) pairs
            def spectrum(nm, lhsT, kdim, rhs):
                big = np_.tile([128, 256], F32, name=nm + "b", tag=nm + "b")
                ps0 = pp.tile([128, 256], F32, name="ps_sp", tag="pss")
                nc.tensor.matmul(ps0[:], lhsT[0:kdim, 0:128], rhs[0:kdim, :],
                                 start=True, stop=True)
                nc.vector.tensor_copy(big[:], ps0[:])
                one = np_.tile([1, 256], F32, name=nm + "o", tag=nm + "o")
                ps1 = pp.tile([1, 256], F32, name="ps_sp1", tag="pss1")
                nc.tensor.matmul(ps1[:], lhsT[0:kdim, 128:129], rhs[0:kdim, :],
                                 start=True, stop=True)
                nc.vector.tensor_copy(one[:], ps1[:])
                return big, one
            irre = spectrum("irre", rnd["gre"], 65, magsT)
            irim = spectrum("irim", rnd["gim"], 65, magsT)
            frre = spectrum("frre", rnd["dre"], 64, frT)
            frim = spectrum("frim", rnd["dim"], 64, frT)
            # P = FR * IR (complex), f32r
            pres, pims = [], []
            for idx in range(2):
                fre, fim = frre[idx], frim[idx]
                ire, iim = irre[idx], irim[idx]
                rows = fre.shape[0]
                t1 = np_.tile([rows, 256], F32, name="t1", tag=f"t1_{idx}")
                t2_ = np_.tile([rows, 256], F32, name="t2n", tag=f"t2n_{idx}")
                pre = np_.tile([rows, 256], F32R, name="pre", tag=f"pre_{idx}")
                pim = np_.tile([rows, 256], F32R, name="pim", tag=f"pim_{idx}")
                nc.vector.tensor_tensor(t1[:], fre[:], ire[:], OP.mult)
                nc.vector.tensor_tensor(t2_[:], fim[:], iim[:], OP.mult)
                nc.vector.tensor_tensor(pre[:], t1[:], t2_[:], OP.subtract)
                nc.vector.tensor_tensor(t1[:], fre[:], iim[:], OP.mult)
                nc.vector.tensor_tensor(t2_[:], fim[:], ire[:], OP.mult)
                nc.vector.tensor_tensor(pim[:], t1[:], t2_[:], OP.add)
                pres.append(pre)
                pims.append(pim)
            # outfullT [256u (2 chunks), 256t] then transpose to of_dram rows
            for mu in range(2):
                pso = pp.tile([128, 256], F32, name="ps_of", tag="pss")
                nc.tensor.matmul(pso[:], rnd["dire0"][:, mu * 128:(mu + 1) * 128],
                                 pres[0][:], start=True, stop=False)
                nc.tensor.matmul(pso[:], rnd["dire1"][:, mu * 128:(mu + 1) * 128],
                                 pres[1][:], start=False, stop=False)
                nc.tensor.matmul(pso[:], rnd["diim0"][:, mu * 128:(mu + 1) * 128],
                                 pims[0][:], start=False, stop=False)
                nc.tensor.matmul(pso[:], rnd["diim1"][:, mu * 128:(mu + 1) * 128],
                                 pims[1][:], start=False, stop=True)
                ofT = np_.tile([128, 256], F32, name="ofT", tag="ofT")
                nc.vector.tensor_copy(ofT[:], pso[:])
                for th_ in range(2):
                    psb = pp.tile([128, 128], F32, name="ps_tb", tag="pst")
                    nc.tensor.transpose(psb[:], ofT[:, th_ * 128:(th_ + 1) * 128],
                                        eye[:])
                    ofb = np_.tile([128, 128], F32, name="ofb", tag="ofb")
                    nc.vector.tensor_copy(ofb[:], psb[:])
                    row0 = 4 + sc_ * 256 + th_ * 128
                    nc.sync.dma_start(
                        self.scr["of_dram"].ap()
                        [row0:row0 + 128, mu * 128:(mu + 1) * 128], ofb[:])

        # ---- overlap-add + crop + combine with harmonic ----
        OF = self.scr["of_dram"].ap()
        for c in range(NCH):
            r0 = 4 + c * 128
            acc = np_.tile([128, 64], F32, name="acc", tag="acc")
            ta = np_.tile([128, 64], F32, name="ta", tag="ta")
            nc.sync.dma_start(acc[:], OF[r0:r0 + 128, 62:126])
            nc.sync.dma_start(ta[:], OF[r0 - 1:r0 + 127, 126:190])
            nc.vector.tensor_tensor(acc[:], acc[:], ta[:], OP.add)
            ta2 = np_.tile([128, 64], F32, name="ta2", tag="ta")
            nc.sync.dma_start(ta2[:], OF[r0 - 2:r0 + 126, 190:254])
            nc.vector.tensor_tensor(acc[:], acc[:], ta2[:], OP.add)
            ta3 = np_.tile([128, 2], F32, name="ta3", tag="tas")
            nc.sync.dma_start(ta3[:], OF[r0 - 3:r0 + 125, 254:256])
            nc.vector.tensor_tensor(acc[:, 0:2], acc[:, 0:2], ta3[:], OP.add)
            ta4 = np_.tile([128, 62], F32, name="ta4", tag="tas")
            nc.sync.dma_start(ta4[:], OF[r0 + 1:r0 + 129, 0:62])
            nc.vector.tensor_tensor(acc[:, 2:64], acc[:, 2:64], ta4[:], OP.add)
            RH = self.scr["rh_dram"].ap()
            r0F = np_.tile([128, 64], F32, name="r0F", tag="r0F")
            r1F = np_.tile([128, 64], F32, name="r1F", tag="r1F")
            nc.sync.dma_start(r0F[:], RH[c * 8192:(c + 1) * 8192]
                              .rearrange("(p j) -> p j", j=64))
            nc.sync.dma_start(r1F[:], RH[NCH * 8192 + c * 8192:
                                         NCH * 8192 + (c + 1) * 8192]
                              .rearrange("(p j) -> p j", j=64))
            hw0 = np_.tile([128, 64], F32, name="hw0", tag="hw", bufs=2)
            nc.vector.tensor_tensor(hw0[:], r0F[:], w0j[:], OP.mult)
            nc.vector.tensor_tensor(acc[:], acc[:], hw0[:], OP.add)
            hw1 = np_.tile([128, 64], F32, name="hw1", tag="hw", bufs=2)
            nc.vector.tensor_tensor(hw1[:], r1F[:], w1j[:], OP.mult)
            nc.vector.tensor_tensor(acc[:], acc[:], hw1[:], OP.add)
            nc.sync.dma_start(self.scr["audio_dram"].ap()
                              [c * 8192:(c + 1) * 8192]
